# revision 50
# baseline (speedup 1.0000x reference)
"""Complex-valued relative-position attention (nn_CAttention) on 8 TRN2 cores.

Sharding: batch (4) x head-half (2) -> 8 cores. Each core computes its
batch's projections for its 4 heads, full attention for those heads, and a
row-split partial output projection. Host sums the two partial outputs per
batch, adds the output bias, and restacks.

Design (v6, ~232us vs v3's 239.5us):
  - Skew-add on the PE: the qrel skew values are accumulated into the dots
    PSUM by fp8 DoubleRow identity matmuls; the readback DMA lands the
    diagonal band directly in [64, ko=2, 1024] DoubleRow layout (part r).
    Most tiles (mode "b") read part i in natural [128,1024] layout and fold
    it via one ADDSQ; every ~5th tile (mode "a") instead identity-adds part
    i in PSUM and squares it on ACT, balancing ACT vs DVE. A single SQACC
    (ei + dpsr^2, one PSUM operand each - the DVE PSUM-port limit) yields
    m2.
  - rel clip regions: columns e<=511 / e>=1535 of the reversed rel table
    are constant, so only the varying ~832-col span plus 2 const columns
    are computed and staged; GPSIMD broadcasts the const columns into the
    fp8 qe tile before the slot write (28% less qrel staging on ACT/DVE
    and 22% fewer qrel matmul columns).
  - dots_i uses a second stationary A_i = [qi*s; qr*s] derived from A by
    two small SBUF copies, replacing the Kni2 staging copies.
  - Vpp is one [128, 8, 512] tile (one copy per v unit); AV output lands
    in per-head OT tiles with a single [128,256] copy (wo is host-permuted
    to head-major rows to match).
  - Softmax: Sqrt in-place on m2, Exp (ACT, batched per 8 tiles for table
    amortization); rowsums accumulate into a shared [128,8] tile, one
    batched reciprocal per group; attn scaling runs on DVE (4x mode).
"""
import functools
import numpy as np

import concourse.bass as bass
import concourse.bacc as bacc
import concourse.mybir as mybir
import concourse.tile as tile
from concourse.bass_utils import run_bass_kernel_spmd

F32 = mybir.dt.float32
F16 = mybir.dt.float16
F8 = mybir.dt.float8e4
AF = mybir.ActivationFunctionType
DR = mybir.MatmulPerfMode.DoubleRow

HEADS, DH, MAX_POS = 8, 64, 512
B, N, DIM = 4, 1024, 512
HPC = 4            # heads per core
KT = 4             # dim k-tiles (512/128)
NT = 8             # n tiles (1024/128)
WIN = 1152         # qrel window width (>= 1151)
SCALE = DH ** (-0.5)
PW = 1             # slot write offset (copies at s, write at s+PW)
PR = 2             # skew readback offset
PB = 5             # stage B offset (skew round-trip prefetch distance)
PL = 2             # batch lag beyond PB (tiles fully ready -> no table leak)
PC = 16            # stage C offset (attn ready after batched Exp)
PD = 17            # stage D offset (processes tile PAIRS on odd steps)
SQG = 8            # sqrt/exp table-batching group size
PRIO_BUMP = 250    # batch priority push


def register_mag2():
    from concourse import dve_ops
    from concourse.dve_spec import Spec, Src0, Src1, AluOp, Bin, lower, sq
    from concourse.dve_uop import DveOpSpec

    existing = [op for op in dve_ops.OPS
                if op.name in ("MAG2_ANT", "ADDSQ_ANT", "SQACC_ANT")]
    if len(existing) == 3:
        return existing

    def reg(name, body, ref):
        spec = Spec(body=body, reference=ref)
        opcode = dve_ops._CUSTOM_DVE_ROW_BASE + len(dve_ops.OPS)
        shas = {}
        for ver in ("v3",):
            s = DveOpSpec(name=name, opcode=opcode,
                          uops=lower(spec, ver=ver), rd1_en=True)
            shas[ver] = s.sha(ver)
        op = dve_ops.DveOp(name, spec, subdim=False, uops_sha=shas)
        dve_ops._SUB_OPCODE_FOR_NAME[op.name] = opcode
        dve_ops.OPS.append(op)
        dve_ops.CUSTOM_DVE_SPECS[op.name] = op.spec
        return op

    op1 = reg("MAG2_ANT", Bin(AluOp.ADD, sq(Src0), sq(Src1)),
              lambda in0, in1, s0, s1, imm2: (
                  in0.astype(np.float32) ** 2 + in1.astype(np.float32) ** 2))
    op2 = reg("ADDSQ_ANT", sq(Bin(AluOp.ADD, Src0, Src1)),
              lambda in0, in1, s0, s1, imm2: (
                  (in0.astype(np.float32) + in1.astype(np.float32)) ** 2))
    op3 = reg("SQACC_ANT", Bin(AluOp.ADD, Src0, sq(Src1)),
              lambda in0, in1, s0, s1, imm2: (
                  in0.astype(np.float32) + in1.astype(np.float32) ** 2))
    return op1, op2, op3


def c_lo(i_blk):
    return 896 - 128 * i_blk


@functools.cache
def build_module():
    import concourse.tile_utils as tile_utils
    if getattr(tile_utils, "max_sbuf_usage", 0) < 208 * 1024:
        tile_utils.max_sbuf_usage = 208 * 1024

    mag2, addsq, sqacc = register_mag2()
    nc = bacc.Bacc("TRN2", target_bir_lowering=False, debug=False,
                   num_devices=8, dynamic_dma_scratch_size=16384)

    din = {}
    for nm, shape, dt_ in [
        ("xt_r", [DIM, N], F16), ("xt_i", [DIM, N], F16),
        ("wq_a", [DIM, 512], F16), ("wq_b", [DIM, 512], F16),
        ("wk_a", [DIM, 512], F16), ("wk_b", [DIM, 512], F16),
        ("wv_a", [DIM, 512], F16), ("wv_b", [DIM, 512], F16),
        ("wo_re", [DIM, 512], F16), ("wo_im", [DIM, 512], F16),
        ("rel_r", [128, 2048], F16), ("rel_i", [128, 2048], F16),
        ("relc", [128, 4], F16),
        ("smask", [128, 1], F32),
        ("ident2", [64, 256], F8),
    ]:
        din[nm] = nc.dram_tensor(nm, shape, dt_, kind="ExternalInput")
    o_r = nc.dram_tensor("o_r", [DIM, N], F16, kind="ExternalOutput")
    o_i = nc.dram_tensor("o_i", [DIM, N], F16, kind="ExternalOutput")

    with tile.TileContext(nc) as tc:
        with (
            tc.tile_pool(name="const", bufs=1) as cpool,
            tc.tile_pool(name="work", bufs=2) as pw,
            tc.tile_pool(name="psB", bufs=2, space="PSUM") as psB,
            tc.tile_pool(name="psU", bufs=4, space="PSUM") as psU,
            tc.tile_pool(name="dram", bufs=16, space="DRAM") as pdram,
        ):
            # ---------------- constants ----------------
            hengs = (nc.sync, nc.scalar)
            smask = cpool.tile([128, 1], F32, tag="smask")
            nc.sync.dma_start(smask[:], din["smask"][:, :])
            relc = cpool.tile([128, 4], F16, tag="relc")
            nc.sync.dma_start(relc[:], din["relc"][:, :])
            ident2 = cpool.tile([64, 2, 128], F8, tag="ident2")
            nc.scalar.dma_start(
                ident2[:], bass.AP(din["ident2"], 0,
                                   [[256, 64], [128, 2], [1, 128]]))

            # load order tuned so Q(0)'s inputs land first
            xtt = {}
            qd = 0

            def load_xt(nm):
                nonlocal qd
                t = pw.tile([128, 4, 1024], F16, tag="xt", bufs=2, name=nm)
                hengs[qd % 2].dma_start(
                    t[:], bass.AP(din[nm], 0,
                                  [[N, 128], [128 * N, 4], [1, N]]))
                qd += 1
                xtt[nm] = t

            def xt(nm, kt, nh):
                return xtt[nm][:, kt, nh * 512:(nh + 1) * 512]

            def load_w(nm, tag, bufs):
                # one [128, 4, 512] tile per weight tensor, single DMA
                nonlocal qd
                t = pw.tile([128, 4, 512], F16, tag=tag, bufs=bufs,
                            name=nm)
                hengs[qd % 2].dma_start(
                    t[:], bass.AP(din[nm], 0,
                                  [[512, 128], [128 * 512, 4], [1, 512]]))
                qd += 1
                return [t[:, kt, :] for kt in range(KT)]

            wqa = load_w("wq_a", "wl", 4)
            load_xt("xt_r")
            wqb = load_w("wq_b", "wl", 4)
            load_xt("xt_i")
            rel_r = cpool.tile([128, 2048], F16, tag="rel_r")
            nc.sync.dma_start(rel_r[:], din["rel_r"][:, :])
            wka = load_w("wk_a", "wl", 4)
            wkb = load_w("wk_b", "wl", 4)
            rel_i = cpool.tile([128, 2048], F16, tag="rel_i")
            nc.scalar.dma_start(rel_i[:], din["rel_i"][:, :])
            wva = load_w("wv_a", "wv", 2)
            wvb = load_w("wv_b", "wv", 2)
            wo_re = cpool.tile([128, 4, 512], F16, tag="wo_re")
            wo_im = cpool.tile([128, 4, 512], F16, tag="wo_im")
            nc.sync.dma_start(
                wo_re[:], bass.AP(din["wo_re"], 0,
                                  [[512, 128], [128 * 512, 4], [1, 512]]))
            nc.scalar.dma_start(
                wo_im[:], bass.AP(din["wo_im"], 0,
                                  [[512, 128], [128 * 512, 4], [1, 512]]))

            A = [None] * HPC
            Ai = [None] * HPC
            Knat = [None] * HPC
            Vpp = pw.tile([128, 8, 512], F16, tag="vpp", bufs=1,
                          name="Vpp")

            def emit_proj_unit(kind, h, nh):
                wa, wb = (wqa, wqb) if kind == "q" else (wka, wkb)
                hs = slice(h * 128, (h + 1) * 128)
                ns = slice(nh * 512, (nh + 1) * 512)
                if kind == "q" and A[h] is None:
                    A[h] = pw.tile([128, 1024], F16, tag="stk", bufs=12,
                                   name=f"A{h}")
                    Ai[h] = pw.tile([128, 1024], F16, tag="stk", bufs=12,
                                    name=f"Ai{h}")
                if kind == "k" and Knat[h] is None:
                    Knat[h] = pw.tile([128, 1024], F16, tag="stk",
                                      bufs=12, name=f"Knat{h}")
                ps = psU.tile([128, 512], F32, tag="pu",
                              name=f"ps{kind}_{h}_{nh}")
                for kt in range(KT):
                    nc.tensor.matmul(ps[:], wa[kt][:, hs],
                                     xt("xt_r", kt, nh),
                                     start=(kt == 0), stop=False)
                for kt in range(KT):
                    nc.tensor.matmul(ps[:], wb[kt][:, hs],
                                     xt("xt_i", kt, nh),
                                     start=False, stop=(kt == KT - 1))
                if kind == "q":
                    nc.vector.tensor_scalar_mul(A[h][:, ns], ps[:],
                                                smask[:])
                    # A_i = [qi*s; qr*s] from A = [qr*s; -qi*s]
                    nc.vector.tensor_scalar_mul(Ai[h][0:64, ns],
                                                A[h][64:128, ns], -1.0)
                    nc.vector.tensor_copy(Ai[h][64:128, ns],
                                          A[h][0:64, ns])
                else:
                    nc.scalar.copy(Knat[h][:, ns], ps[:])

            def emit_vproj_unit(J):
                xs = slice((J % 4) * 128, (J % 4) * 128 + 128)
                vps = psU.tile([128, 512], F32, tag="pu", name=f"vps_{J}")
                for kt in range(KT):
                    nc.tensor.matmul(vps[:],
                                     xt("xt_r", kt, J // 4)[:, xs],
                                     wva[kt][:, :],
                                     start=(kt == 0), stop=False)
                for kt in range(KT):
                    nc.tensor.matmul(vps[:],
                                     xt("xt_i", kt, J // 4)[:, xs],
                                     wvb[kt][:, :],
                                     start=False, stop=(kt == KT - 1))
                nc.vector.tensor_copy(Vpp[:, J, :], vps[:])

            # head 0 Q/K up front; the rest feeds the loop's PE slack
            for kind in ("q", "k"):
                for nh in range(2):
                    emit_proj_unit(kind, 0, nh)
            punits = [("q", 1, 0), ("q", 1, 1), ("k", 1, 0), ("k", 1, 1),
                      ("v", 0, None), ("v", 1, None), ("v", 2, None),
                      ("v", 3, None),
                      ("q", 2, 0), ("q", 2, 1), ("k", 2, 0), ("k", 2, 1),
                      ("v", 4, None), ("v", 5, None), ("v", 6, None),
                      ("v", 7, None),
                      ("q", 3, 0), ("q", 3, 1), ("k", 3, 0), ("k", 3, 1)]

            # OT stacks: per-head [avr(64); avi(64)] x n, [128, 1024] fp16
            OT = [pw.tile([128, 1024], F16, tag="otk", bufs=4,
                          name=f"OT{t}") for t in range(4)]

            # ---------------- attention pipeline stages ----------------
            # rel columns e in [512, 1534] vary; e<=511 are all rel_emb[2M]
            # and e>=1535 all rel_emb[0]. Only the varying span + 2 const
            # columns are computed/staged; Pool broadcasts the const
            # regions into qe before the slot write.
            def qwin(I):
                lo = c_lo(I)
                head = max(0, 512 - lo)
                tail = max(0, lo + WIN - 1535)
                return head, WIN - head - tail

            def emit_qrel_part(h, I, part, qe, qc):
                isl = slice(I * 128, (I + 1) * 128)
                lo = c_lo(I)
                head, w = qwin(I)
                abs_lo = lo + head
                relt = rel_r if part == 0 else rel_i
                qpss = []
                for ci, (c0, c1) in enumerate(((0, 512), (512, w))):
                    qps = psU.tile([128, c1 - c0], F32, tag="pu",
                                   name=f"qps{part}_{h}_{I}_{ci}")
                    nc.tensor.matmul(qps[:], A[h][:, isl],
                                     relt[:, abs_lo + c0:abs_lo + c1],
                                     start=True, stop=True)
                    qpss.append(qps)
                qcps = psU.tile([128, 2], F32, tag="pu",
                                name=f"qcps{part}_{h}_{I}")
                nc.tensor.matmul(qcps[:], A[h][:, isl],
                                 relc[:, part * 2:part * 2 + 2],
                                 start=True, stop=True)
                for ci, (c0, c1) in enumerate(((0, 512), (512, w))):
                    dst = qe[:, part, head + c0:head + c1]
                    if ci == 0:
                        nc.vector.tensor_copy(dst, qpss[ci][:])
                    else:
                        nc.scalar.copy(dst, qpss[ci][:])
                nc.vector.tensor_copy(qc[:, part, :], qcps[:])

            def emit_qe_bcast(h, I, qe, qc):
                head, w = qwin(I)
                for part in range(2):
                    if head:
                        nc.gpsimd.tensor_copy(
                            qe[:, part, 0:head],
                            qc[:, part, 0:1].broadcast_to([128, head]))
                    if head + w < WIN:
                        nc.gpsimd.tensor_copy(
                            qe[:, part, head + w:WIN],
                            qc[:, part, 1:2].broadcast_to(
                                [128, WIN - head - w]))

            def emit_qrel_write(h, I, qe):
                slot = pdram.tile([128, 2 * WIN], F8, tag="qrev",
                                  name=f"qrev_{h}_{I}")
                nc.gpsimd.dma_start(
                    bass.AP(slot.tensor, 0,
                            [[2 * WIN, 128], [WIN, 2], [1, WIN]]),
                    qe[:])
                return slot

            def emit_qrel_read(h, I, slot, mode):
                # part r in DoubleRow layout: (p, ko, j) <- slot row
                # 2p+ko, band col 127-(2p+ko)+j
                if mode == "a":
                    skw = pw.tile([64, 2, 2, 1024], F8, tag="skw", bufs=2,
                                  name=f"skew_{h}_{I}")
                    nc.sync.dma_start(
                        skw[:],
                        bass.AP(slot.tensor, 127,
                                [[2 * (2 * WIN - 1), 64], [2 * WIN - 1, 2],
                                 [WIN, 2], [1, 1024]]))
                    return skw, None
                skwr = pw.tile([64, 2, 1024], F8, tag="skwr", bufs=6,
                               name=f"skewr_{h}_{I}")
                nc.sync.dma_start(
                    skwr[:],
                    bass.AP(slot.tensor, 127,
                            [[2 * (2 * WIN - 1), 64], [2 * WIN - 1, 2],
                             [1, 1024]]))
                # part i natural: (p, j) <- slot row p, col W+127-p+j
                skwi = pw.tile([128, 1024], F8, tag="skwi", bufs=6,
                               name=f"skewi_{h}_{I}")
                nc.scalar.dma_start(
                    skwi[:],
                    bass.AP(slot.tensor, WIN + 127,
                            [[2 * WIN - 1, 128], [1, 1024]]))
                return skwr, skwi

            def stage_B(h, I, skws, mode):
                skwr, skwi = skws
                isl = slice(I * 128, (I + 1) * 128)
                # i part first: its SBUF crossing overlaps the r matmuls
                dpsi = psB.tile([128, 1024], F32, tag="pb",
                                name=f"dpsi_{h}_{I}")
                for nh in range(2):
                    ns = slice(nh * 512, (nh + 1) * 512)
                    nc.tensor.matmul(dpsi[:, ns], Ai[h][:, isl],
                                     Knat[h][:, ns], start=True,
                                     stop=(mode == "b"))
                    if mode == "a":
                        nc.tensor.matmul(dpsi[:, ns], ident2[:],
                                         skwr[:, :, 1, ns], start=False,
                                         stop=True, perf_mode=DR)
                ui = pw.tile([128, 1024], F16, tag="ui", bufs=3,
                             name=f"ui_{h}_{I}")
                if mode == "a":
                    nc.scalar.activation(ui[:], dpsi[:], AF.Square)
                else:
                    nc.vector._custom_dve(addsq, out=ui[:],
                                          in0=skwi[:], in1=dpsi[:])
                dpsr = psB.tile([128, 1024], F32, tag="pb",
                                name=f"dpsr_{h}_{I}")
                for nh in range(2):
                    ns = slice(nh * 512, (nh + 1) * 512)
                    nc.tensor.matmul(dpsr[:, ns], A[h][:, isl],
                                     Knat[h][:, ns], start=True, stop=False)
                    sk = skwr[:, :, 0, ns] if mode == "a" else skwr[:, :, ns]
                    nc.tensor.matmul(dpsr[:, ns], ident2[:], sk,
                                     start=False, stop=True, perf_mode=DR)
                m2 = pw.tile([128, 1024], F16, tag="m2", bufs=12,
                             name=f"m2_{h}_{I}")
                nc.vector._custom_dve(sqacc, out=m2[:],
                                      in0=ui[:], in1=dpsr[:])
                return m2

            def emit_sqrt(h, I, m2):
                # in-place: mag overwrites m2
                nc.scalar.activation(m2[:], m2[:], AF.Sqrt)
                return m2

            def emit_exp(h, I, mag, rs8, col):
                attn = pw.tile([128, 1024], F16, tag="attn", bufs=10,
                               name=f"attn_{h}_{I}")
                nc.scalar.activation(attn[:], mag[:], AF.Exp,
                                     accum_out=rs8[:, col:col + 1])
                return attn

            def stage_C1(h, I, attn, rc8, col):
                nc.vector.tensor_scalar_mul(attn[:], attn[:],
                                            rc8[:, col:col + 1])
                return attn

            def stage_C2(h, I, attn, atP):
                half = slice((I % 2) * 128, (I % 2) * 128 + 128)
                nc.sync.dma_start(atP[:, :, half], attn[:], transpose=True)

            def stage_D_pair(h, I0, atP):
                # tiles (h, I0) and (h, I0+1) share one AV matmul pass
                isl = slice(I0 * 128, (I0 + 2) * 128)
                avs = psU.tile([128, 256], F32, tag="pu",
                               name=f"avs_{h}_{I0}")
                vsl = slice(h * 128, (h + 1) * 128)
                for J in range(NT):
                    nc.tensor.matmul(avs[:], Vpp[:, J, vsl],
                                     atP[:, J, :],
                                     start=(J == 0), stop=(J == NT - 1))
                nc.vector.tensor_copy(OT[h][:, isl], avs[:])

            def emit_outproj(nh):
                ns = slice(nh * 512, (nh + 1) * 512)
                for part, wo_s in ((0, wo_re), (1, wo_im)):
                    for dt_ in range(4):
                        ds = slice(dt_ * 128, (dt_ + 1) * 128)
                        ops = psU.tile([128, 512], F32, tag="pu",
                                       name=f"ops_{part}_{dt_}_{nh}")
                        for j in range(4):
                            nc.tensor.matmul(ops[:], wo_s[:, j, ds],
                                             OT[j][:, ns],
                                             start=(j == 0), stop=(j == 3))
                        osb = pw.tile([128, 512], F16, tag="osb", bufs=3,
                                      name=f"osb_{part}_{dt_}_{nh}")
                        nc.scalar.copy(osb[:], ops[:])
                        dst = o_r if part == 0 else o_i
                        nc.sync.dma_start(
                            bass.AP(dst, dt_ * 128 * N + nh * 512,
                                    [[N, 128], [1, 512]]),
                            osb[:])

            flat = [(h, I) for h in range(HPC) for I in range(NT)]
            NF = len(flat)
            (qe_map, qe_done, slotmap, skewmap, m2map, magmap, attnmap,
             atPmap) = ({} for _ in range(8))
            rs8map, rc8map = {}, {}
            for s in range(NF + PD + 1):
                if punits:
                    kind, a1, a2 = punits.pop(0)
                    if kind == "v":
                        emit_vproj_unit(a1)
                    else:
                        emit_proj_unit(kind, a1, a2)
                if s < NF:
                    h, I = flat[s]
                    qe_map[(h, I)] = (
                        pw.tile([128, 2, WIN], F8, tag="qe",
                                bufs=6, name=f"qe_{h}_{I}"),
                        pw.tile([128, 2, 2], F8, tag="qc",
                                bufs=6, name=f"qc_{h}_{I}"))
                    emit_qrel_part(h, I, 0, *qe_map[(h, I)])
                if PW <= s < NF + PW:
                    h, I = flat[s - PW]
                    slotmap[(h, I)] = emit_qrel_write(h, I,
                                                      qe_done.pop((h, I)))
                if PR <= s < NF + PR:
                    h, I = flat[s - PR]
                    skewmap[(h, I)] = emit_qrel_read(
                        h, I, slotmap.pop((h, I)),
                        "a" if (s - PR) % 5 == 2 else "b")
                # batched Sqrt+Exp (SQG tiles), lagged PL iterations;
                # priority-pushed so later iterations' table-neutral copies
                # interleave instead of stalling behind the burst
                t = s - PB - PL
                if 0 <= t < NF and t % SQG == SQG - 1:
                    g = t // SQG
                    prio0 = tc.cur_priority
                    tc.cur_priority = prio0 + PRIO_BUMP
                    for tt in range(t - SQG + 1, t + 1):
                        hh, ii = flat[tt]
                        magmap[(hh, ii)] = emit_sqrt(hh, ii,
                                                     m2map.pop((hh, ii)))
                    rs8 = pw.tile([128, SQG], F32, tag="sm", bufs=3,
                                  name=f"rs8_{g}")
                    rs8map[g] = rs8
                    for tt in range(t - SQG + 1, t + 1):
                        hh, ii = flat[tt]
                        attnmap[(hh, ii)] = emit_exp(hh, ii,
                                                     magmap.pop((hh, ii)),
                                                     rs8, tt % SQG)
                    rc8 = pw.tile([128, SQG], F32, tag="sm", bufs=3,
                                  name=f"rc8_{g}")
                    nc.vector.reciprocal(rc8[:], rs8[:])
                    rc8map[g] = rc8
                    tc.cur_priority = prio0
                if PC - 1 <= s < NF + PC - 1:
                    h, I = flat[s - PC + 1]
                    t1 = s - PC + 1
                    attnmap[(h, I)] = stage_C1(h, I, attnmap.pop((h, I)),
                                               rc8map[t1 // SQG], t1 % SQG)
                if PC <= s < NF + PC:
                    h, I = flat[s - PC]
                    if I % 2 == 0:
                        atPmap[(h, I // 2)] = pw.tile(
                            [128, 8, 256], F16, tag="att", bufs=3,
                            name=f"atP_{h}_{I // 2}")
                    stage_C2(h, I, attnmap.pop((h, I)), atPmap[(h, I // 2)])
                if PD <= s < NF + PD and (s - PD) % 2 == 1:
                    h, I = flat[s - PD]
                    stage_D_pair(h, I - 1, atPmap.pop((h, I // 2)))
                    if (h, I) == (HPC - 1, 3):
                        emit_outproj(0)
                if PB <= s < NF + PB:
                    h, I = flat[s - PB]
                    m2map[(h, I)] = stage_B(h, I, skewmap.pop((h, I)),
                                            "a" if (s - PB) % 5 == 2
                                            else "b")
                if s < NF:
                    h, I = flat[s]
                    qe, qc = qe_map.pop((h, I))
                    emit_qrel_part(h, I, 1, qe, qc)
                    emit_qe_bcast(h, I, qe, qc)
                    qe_done[(h, I)] = qe
            emit_outproj(1)

    nc.compile()
    return nc, mag2


def _prep_core_inputs(inputs, core):
    import ml_dtypes
    b, half = core // 2, core % 2
    x = inputs["x"]
    f16 = np.float16
    xt_r = np.ascontiguousarray(x[b, :, :, 0].T).astype(f16)
    xt_i = np.ascontiguousarray(x[b, :, :, 1].T).astype(f16)

    def pack_ab(wr, wi):
        a = np.empty((DIM, 512), f16)
        bb = np.empty((DIM, 512), f16)
        for hl in range(HPC):
            gh = half * HPC + hl
            cs = slice(gh * DH, (gh + 1) * DH)
            a[:, hl * 128:hl * 128 + 64] = wr[:, cs]
            a[:, hl * 128 + 64:hl * 128 + 128] = wi[:, cs]
            bb[:, hl * 128:hl * 128 + 64] = -wi[:, cs]
            bb[:, hl * 128 + 64:hl * 128 + 128] = wr[:, cs]
        return a, bb

    wq_a, wq_b = pack_ab(inputs["wq_r"], inputs["wq_i"])
    wk_a, wk_b = pack_ab(inputs["wkv_r"][:, :512], inputs["wkv_i"][:, :512])
    wv_a, wv_b = pack_ab(inputs["wkv_r"][:, 512:], inputs["wkv_i"][:, 512:])

    # wo rows permuted head-major: per head hl, rows [r(64); i-part(64)]
    rs0 = half * 256
    wo_re = np.empty((DIM, 512), f16)
    wo_im = np.empty((DIM, 512), f16)
    for hl in range(HPC):
        rr = slice(rs0 + hl * 64, rs0 + (hl + 1) * 64)
        dst_r = slice(hl * 128, hl * 128 + 64)
        dst_i = slice(hl * 128 + 64, hl * 128 + 128)
        wo_re[dst_r] = inputs["wo_r"][rr, :]
        wo_re[dst_i] = -inputs["wo_i"][rr, :]
        wo_im[dst_r] = inputs["wo_i"][rr, :]
        wo_im[dst_i] = inputs["wo_r"][rr, :]

    e = np.arange(2047)
    t_ext = inputs["rel_emb"][np.clip(e - 1023, -MAX_POS, MAX_POS) + MAX_POS]
    relrev = t_ext[::-1].astype(np.float32)      # [2047, 64]
    rel_r = np.zeros((128, 2048), f16)
    rel_i = np.zeros((128, 2048), f16)
    rel_r[0:64, 0:2047] = relrev.T.astype(f16)
    rel_i[64:128, 0:2047] = (-relrev.T).astype(f16)
    # const columns: [r_head(e=511), r_tail(e=1535), i_head, i_tail]
    relc = np.zeros((128, 4), f16)
    relc[:, 0] = rel_r[:, 511]
    relc[:, 1] = rel_r[:, 1535]
    relc[:, 2] = rel_i[:, 511]
    relc[:, 3] = rel_i[:, 1535]

    smask = np.concatenate(
        [np.full(64, SCALE, np.float32),
         np.full(64, -SCALE, np.float32)]).reshape(128, 1)

    ident2 = np.zeros((64, 2, 128), np.float32)
    for p in range(64):
        for k in range(2):
            ident2[p, k, 2 * p + k] = 1.0
    ident2 = ident2.reshape(64, 256).astype(ml_dtypes.float8_e4m3)

    return {
        "xt_r": xt_r, "xt_i": xt_i,
        "wq_a": wq_a, "wq_b": wq_b, "wk_a": wk_a, "wk_b": wk_b,
        "wv_a": wv_a, "wv_b": wv_b, "wo_re": wo_re, "wo_im": wo_im,
        "rel_r": rel_r, "rel_i": rel_i, "relc": relc, "smask": smask,
        "ident2": ident2,
    }


_last_results = {}


def kernel(**inputs):
    inputs = {k: np.asarray(v) for k, v in inputs.items()}
    nc, _ = build_module()
    in_maps = [_prep_core_inputs(inputs, c) for c in range(8)]
    res = run_bass_kernel_spmd(nc, in_maps, core_ids=list(range(8)))
    _last_results["res"] = res

    bo_r = inputs["bo_r"].astype(np.float32)
    bo_i = inputs["bo_i"].astype(np.float32)
    out = np.empty((B, N, DIM, 2), np.float32)
    for b in range(B):
        r = (res.results[2 * b]["o_r"].astype(np.float32)
             + res.results[2 * b + 1]["o_r"].astype(np.float32))
        i = (res.results[2 * b]["o_i"].astype(np.float32)
             + res.results[2 * b + 1]["o_i"].astype(np.float32))
        out[b, :, :, 0] = r.T + bo_r[None, :]
        out[b, :, :, 1] = i.T + bo_i[None, :]
    return out


# revision 52
# speedup vs baseline: 1.0151x; 1.0151x over previous
"""Complex-valued relative-position attention (nn_CAttention) on 8 TRN2 cores.

Sharding: batch (4) x head-half (2) -> 8 cores. Each core computes its
batch's projections for its 4 heads, full attention for those heads, and a
row-split partial output projection. Host sums the two partial outputs per
batch, adds the output bias, and restacks.

Design (v6, ~232us vs v3's 239.5us):
  - Skew-add on the PE: the qrel skew values are accumulated into the dots
    PSUM by fp8 DoubleRow identity matmuls; the readback DMA lands the
    diagonal band directly in [64, ko=2, 1024] DoubleRow layout (part r).
    Most tiles (mode "b") read part i in natural [128,1024] layout and fold
    it via one ADDSQ; every ~5th tile (mode "a") instead identity-adds part
    i in PSUM and squares it on ACT, balancing ACT vs DVE. A single SQACC
    (ei + dpsr^2, one PSUM operand each - the DVE PSUM-port limit) yields
    m2.
  - rel clip regions: columns e<=511 / e>=1535 of the reversed rel table
    are constant, so only the varying ~832-col span plus 2 const columns
    are computed and staged; GPSIMD broadcasts the const columns into the
    fp8 qe tile before the slot write (28% less qrel staging on ACT/DVE
    and 22% fewer qrel matmul columns).
  - dots_i uses a second stationary A_i = [qi*s; qr*s] derived from A by
    two small SBUF copies, replacing the Kni2 staging copies.
  - Vpp is one [128, 8, 512] tile (one copy per v unit); AV output lands
    in per-head OT tiles with a single [128,256] copy (wo is host-permuted
    to head-major rows to match).
  - Softmax: Sqrt in-place on m2, Exp (ACT, batched per 8 tiles for table
    amortization); rowsums accumulate into a shared [128,8] tile, one
    batched reciprocal per group; attn scaling runs on DVE (4x mode).
"""
import functools
import numpy as np

import concourse.bass as bass
import concourse.bacc as bacc
import concourse.mybir as mybir
import concourse.tile as tile
from concourse.bass_utils import run_bass_kernel_spmd

F32 = mybir.dt.float32
F16 = mybir.dt.float16
F8 = mybir.dt.float8e4
AF = mybir.ActivationFunctionType
DR = mybir.MatmulPerfMode.DoubleRow

HEADS, DH, MAX_POS = 8, 64, 512
B, N, DIM = 4, 1024, 512
HPC = 4            # heads per core
KT = 4             # dim k-tiles (512/128)
NT = 8             # n tiles (1024/128)
WIN = 1152         # qrel window width (>= 1151)
SCALE = DH ** (-0.5)
PW = 1             # slot write offset (copies at s, write at s+PW)
PR = 2             # skew readback offset
PB = 5             # stage B offset (skew round-trip prefetch distance)
PL = 2             # batch lag beyond PB (tiles fully ready -> no table leak)
PC = 16            # stage C offset (attn ready after batched Exp)
PD = 17            # stage D offset (processes tile PAIRS on odd steps)
SQG = 8            # sqrt/exp table-batching group size
PRIO_BUMP = 250    # batch priority push


def register_mag2():
    from concourse import dve_ops
    from concourse.dve_spec import Spec, Src0, Src1, AluOp, Bin, lower, sq
    from concourse.dve_uop import DveOpSpec

    existing = [op for op in dve_ops.OPS
                if op.name in ("MAG2_ANT", "ADDSQ_ANT", "SQACC_ANT")]
    if len(existing) == 3:
        return existing

    def reg(name, body, ref):
        spec = Spec(body=body, reference=ref)
        opcode = dve_ops._CUSTOM_DVE_ROW_BASE + len(dve_ops.OPS)
        shas = {}
        for ver in ("v3",):
            s = DveOpSpec(name=name, opcode=opcode,
                          uops=lower(spec, ver=ver), rd1_en=True)
            shas[ver] = s.sha(ver)
        op = dve_ops.DveOp(name, spec, subdim=False, uops_sha=shas)
        dve_ops._SUB_OPCODE_FOR_NAME[op.name] = opcode
        dve_ops.OPS.append(op)
        dve_ops.CUSTOM_DVE_SPECS[op.name] = op.spec
        return op

    op1 = reg("MAG2_ANT", Bin(AluOp.ADD, sq(Src0), sq(Src1)),
              lambda in0, in1, s0, s1, imm2: (
                  in0.astype(np.float32) ** 2 + in1.astype(np.float32) ** 2))
    op2 = reg("ADDSQ_ANT", sq(Bin(AluOp.ADD, Src0, Src1)),
              lambda in0, in1, s0, s1, imm2: (
                  (in0.astype(np.float32) + in1.astype(np.float32)) ** 2))
    op3 = reg("SQACC_ANT", Bin(AluOp.ADD, Src0, sq(Src1)),
              lambda in0, in1, s0, s1, imm2: (
                  in0.astype(np.float32) + in1.astype(np.float32) ** 2))
    return op1, op2, op3


def c_lo(i_blk):
    return 896 - 128 * i_blk


@functools.cache
def build_module():
    import concourse.tile_utils as tile_utils
    if getattr(tile_utils, "max_sbuf_usage", 0) < 208 * 1024:
        tile_utils.max_sbuf_usage = 208 * 1024

    mag2, addsq, sqacc = register_mag2()
    nc = bacc.Bacc("TRN2", target_bir_lowering=False, debug=False,
                   num_devices=8, dynamic_dma_scratch_size=16384)

    din = {}
    for nm, shape, dt_ in [
        ("xt_r", [DIM, N], F16), ("xt_i", [DIM, N], F16),
        ("wq_a", [DIM, 512], F16), ("wq_b", [DIM, 512], F16),
        ("wk_a", [DIM, 512], F16), ("wk_b", [DIM, 512], F16),
        ("wv_a", [DIM, 512], F16), ("wv_b", [DIM, 512], F16),
        ("wo_re", [DIM, 512], F16), ("wo_im", [DIM, 512], F16),
        ("rel_r", [128, 2048], F16), ("rel_i", [128, 2048], F16),
        ("smask", [128, 1], F32),
        ("ident2", [64, 256], F8),
    ]:
        din[nm] = nc.dram_tensor(nm, shape, dt_, kind="ExternalInput")
    o_r = nc.dram_tensor("o_r", [DIM, N], F16, kind="ExternalOutput")
    o_i = nc.dram_tensor("o_i", [DIM, N], F16, kind="ExternalOutput")

    with tile.TileContext(nc) as tc:
        with (
            tc.tile_pool(name="const", bufs=1) as cpool,
            tc.tile_pool(name="work", bufs=2) as pw,
            tc.tile_pool(name="psB", bufs=2, space="PSUM") as psB,
            tc.tile_pool(name="psU", bufs=4, space="PSUM") as psU,
            tc.tile_pool(name="dram", bufs=16, space="DRAM") as pdram,
        ):
            # ---------------- constants ----------------
            hengs = (nc.sync, nc.scalar)
            smask = cpool.tile([128, 1], F32, tag="smask")
            nc.sync.dma_start(smask[:], din["smask"][:, :])
            ident2 = cpool.tile([64, 2, 128], F8, tag="ident2")
            nc.scalar.dma_start(
                ident2[:], bass.AP(din["ident2"], 0,
                                   [[256, 64], [128, 2], [1, 128]]))

            # load order tuned so Q(0)'s inputs land first
            xtt = {}
            qd = 0

            def load_xt(nm):
                nonlocal qd
                t = pw.tile([128, 4, 1024], F16, tag="xt", bufs=2, name=nm)
                hengs[qd % 2].dma_start(
                    t[:], bass.AP(din[nm], 0,
                                  [[N, 128], [128 * N, 4], [1, N]]))
                qd += 1
                xtt[nm] = t

            def xt(nm, kt, nh):
                return xtt[nm][:, kt, nh * 512:(nh + 1) * 512]

            def load_w(nm, tag, bufs):
                # one [128, 4, 512] tile per weight tensor, single DMA
                nonlocal qd
                t = pw.tile([128, 4, 512], F16, tag=tag, bufs=bufs,
                            name=nm)
                hengs[qd % 2].dma_start(
                    t[:], bass.AP(din[nm], 0,
                                  [[512, 128], [128 * 512, 4], [1, 512]]))
                qd += 1
                return [t[:, kt, :] for kt in range(KT)]

            wqa = load_w("wq_a", "wl", 4)
            load_xt("xt_r")
            wqb = load_w("wq_b", "wl", 4)
            load_xt("xt_i")
            rel_r = cpool.tile([128, 2048], F16, tag="rel_r")
            nc.sync.dma_start(rel_r[:], din["rel_r"][:, :])
            wka = load_w("wk_a", "wl", 4)
            wkb = load_w("wk_b", "wl", 4)
            rel_i = cpool.tile([128, 2048], F16, tag="rel_i")
            nc.scalar.dma_start(rel_i[:], din["rel_i"][:, :])
            wva = load_w("wv_a", "wv", 2)
            wvb = load_w("wv_b", "wv", 2)
            wo_re = cpool.tile([128, 4, 512], F16, tag="wo_re")
            wo_im = cpool.tile([128, 4, 512], F16, tag="wo_im")
            nc.sync.dma_start(
                wo_re[:], bass.AP(din["wo_re"], 0,
                                  [[512, 128], [128 * 512, 4], [1, 512]]))
            nc.scalar.dma_start(
                wo_im[:], bass.AP(din["wo_im"], 0,
                                  [[512, 128], [128 * 512, 4], [1, 512]]))

            A = [None] * HPC
            Ai = [None] * HPC
            Knat = [None] * HPC
            Vpp = pw.tile([128, 8, 512], F16, tag="vpp", bufs=1,
                          name="Vpp")

            def emit_proj_unit(kind, h, nh):
                wa, wb = (wqa, wqb) if kind == "q" else (wka, wkb)
                hs = slice(h * 128, (h + 1) * 128)
                ns = slice(nh * 512, (nh + 1) * 512)
                if kind == "q" and A[h] is None:
                    A[h] = pw.tile([128, 1024], F16, tag="stk", bufs=12,
                                   name=f"A{h}")
                    Ai[h] = pw.tile([128, 1024], F16, tag="stk", bufs=12,
                                    name=f"Ai{h}")
                if kind == "k" and Knat[h] is None:
                    Knat[h] = pw.tile([128, 1024], F16, tag="stk",
                                      bufs=12, name=f"Knat{h}")
                ps = psU.tile([128, 512], F32, tag="pu",
                              name=f"ps{kind}_{h}_{nh}")
                for kt in range(KT):
                    nc.tensor.matmul(ps[:], wa[kt][:, hs],
                                     xt("xt_r", kt, nh),
                                     start=(kt == 0), stop=False)
                for kt in range(KT):
                    nc.tensor.matmul(ps[:], wb[kt][:, hs],
                                     xt("xt_i", kt, nh),
                                     start=False, stop=(kt == KT - 1))
                if kind == "q":
                    nc.vector.tensor_scalar_mul(A[h][:, ns], ps[:],
                                                smask[:])
                    # A_i = [qi*s; qr*s] from A = [qr*s; -qi*s]
                    nc.vector.tensor_scalar_mul(Ai[h][0:64, ns],
                                                A[h][64:128, ns], -1.0)
                    nc.vector.tensor_copy(Ai[h][64:128, ns],
                                          A[h][0:64, ns])
                else:
                    nc.scalar.copy(Knat[h][:, ns], ps[:])

            def emit_vproj_unit(J):
                xs = slice((J % 4) * 128, (J % 4) * 128 + 128)
                vps = psU.tile([128, 512], F32, tag="pu", name=f"vps_{J}")
                for kt in range(KT):
                    nc.tensor.matmul(vps[:],
                                     xt("xt_r", kt, J // 4)[:, xs],
                                     wva[kt][:, :],
                                     start=(kt == 0), stop=False)
                for kt in range(KT):
                    nc.tensor.matmul(vps[:],
                                     xt("xt_i", kt, J // 4)[:, xs],
                                     wvb[kt][:, :],
                                     start=False, stop=(kt == KT - 1))
                nc.vector.tensor_copy(Vpp[:, J, :], vps[:])

            # head 0 Q/K up front; the rest feeds the loop's PE slack
            for kind in ("q", "k"):
                for nh in range(2):
                    emit_proj_unit(kind, 0, nh)
            punits = [("q", 1, 0), ("q", 1, 1), ("k", 1, 0), ("k", 1, 1),
                      ("v", 0, None), ("v", 1, None), ("v", 2, None),
                      ("v", 3, None),
                      ("q", 2, 0), ("q", 2, 1), ("k", 2, 0), ("k", 2, 1),
                      ("v", 4, None), ("v", 5, None), ("v", 6, None),
                      ("v", 7, None),
                      ("q", 3, 0), ("q", 3, 1), ("k", 3, 0), ("k", 3, 1)]

            # OT stacks: per-head [avr(64); avi(64)] x n, [128, 1024] fp16
            OT = [pw.tile([128, 1024], F16, tag="otk", bufs=4,
                          name=f"OT{t}") for t in range(4)]

            # ---------------- attention pipeline stages ----------------
            # rel columns e in [512, 1534] vary; e<=511 are all rel_emb[2M]
            # and e>=1535 all rel_emb[0]. Only the varying span + 2 const
            # columns are computed/staged; Pool broadcasts the const
            # regions into qe before the slot write.
            def qwin(I):
                lo = c_lo(I)
                head = max(0, 512 - lo)
                tail = max(0, lo + WIN - 1535)
                return head, WIN - head - tail

            def emit_qrel_part(h, I, part, qe):
                # The const columns (e=511 head / e=1535 tail) adjoin the
                # varying span in e-space, so they ride inside the chunk
                # matmuls; the broadcasts later replicate them in qe.
                isl = slice(I * 128, (I + 1) * 128)
                lo = c_lo(I)
                head, w = qwin(I)
                tail = WIN - head - w
                e0 = lo + head - (1 if head else 0)
                W = w + (1 if head else 0) + (1 if tail else 0)
                q0 = head - (1 if head else 0)
                relt = rel_r if part == 0 else rel_i
                chunks = []
                c = 0
                while c < W:
                    chunks.append((c, min(c + 512, W)))
                    c += 512
                qpss = []
                for ci, (c0, c1) in enumerate(chunks):
                    qps = psU.tile([128, c1 - c0], F32, tag="pu",
                                   name=f"qps{part}_{h}_{I}_{ci}")
                    nc.tensor.matmul(qps[:], A[h][:, isl],
                                     relt[:, e0 + c0:e0 + c1],
                                     start=True, stop=True)
                    qpss.append(qps)
                for ci, (c0, c1) in enumerate(chunks):
                    dst = qe[:, part, q0 + c0:q0 + c1]
                    if ci == 0:
                        nc.vector.tensor_copy(dst, qpss[ci][:])
                    else:
                        nc.scalar.copy(dst, qpss[ci][:])

            def emit_qe_bcast(h, I, qe):
                head, w = qwin(I)
                for part in range(2):
                    if head > 1:
                        nc.gpsimd.tensor_copy(
                            qe[:, part, 0:head - 1],
                            qe[:, part, head - 1:head].broadcast_to(
                                [128, head - 1]))
                    if head + w + 1 < WIN:
                        nc.gpsimd.tensor_copy(
                            qe[:, part, head + w + 1:WIN],
                            qe[:, part, head + w:head + w + 1].broadcast_to(
                                [128, WIN - head - w - 1]))

            def emit_qrel_write(h, I, qe):
                slot = pdram.tile([128, 2 * WIN], F8, tag="qrev",
                                  name=f"qrev_{h}_{I}")
                nc.gpsimd.dma_start(
                    bass.AP(slot.tensor, 0,
                            [[2 * WIN, 128], [WIN, 2], [1, WIN]]),
                    qe[:])
                return slot

            def emit_qrel_read(h, I, slot, mode):
                # part r in DoubleRow layout: (p, ko, j) <- slot row
                # 2p+ko, band col 127-(2p+ko)+j
                if mode == "a":
                    skw = pw.tile([64, 2, 2, 1024], F8, tag="skw", bufs=2,
                                  name=f"skew_{h}_{I}")
                    nc.sync.dma_start(
                        skw[:],
                        bass.AP(slot.tensor, 127,
                                [[2 * (2 * WIN - 1), 64], [2 * WIN - 1, 2],
                                 [WIN, 2], [1, 1024]]))
                    return skw, None
                skwr = pw.tile([64, 2, 1024], F8, tag="skwr", bufs=6,
                               name=f"skewr_{h}_{I}")
                nc.sync.dma_start(
                    skwr[:],
                    bass.AP(slot.tensor, 127,
                            [[2 * (2 * WIN - 1), 64], [2 * WIN - 1, 2],
                             [1, 1024]]))
                # part i natural: (p, j) <- slot row p, col W+127-p+j
                skwi = pw.tile([128, 1024], F8, tag="skwi", bufs=6,
                               name=f"skewi_{h}_{I}")
                nc.scalar.dma_start(
                    skwi[:],
                    bass.AP(slot.tensor, WIN + 127,
                            [[2 * WIN - 1, 128], [1, 1024]]))
                return skwr, skwi

            def stage_B(h, I, skws, mode):
                skwr, skwi = skws
                isl = slice(I * 128, (I + 1) * 128)
                # i part first: its SBUF crossing overlaps the r matmuls
                dpsi = psB.tile([128, 1024], F32, tag="pb",
                                name=f"dpsi_{h}_{I}")
                for nh in range(2):
                    ns = slice(nh * 512, (nh + 1) * 512)
                    nc.tensor.matmul(dpsi[:, ns], Ai[h][:, isl],
                                     Knat[h][:, ns], start=True,
                                     stop=(mode == "b"))
                    if mode == "a":
                        nc.tensor.matmul(dpsi[:, ns], ident2[:],
                                         skwr[:, :, 1, ns], start=False,
                                         stop=True, perf_mode=DR)
                ui = pw.tile([128, 1024], F16, tag="ui", bufs=3,
                             name=f"ui_{h}_{I}")
                if mode == "a":
                    nc.scalar.activation(ui[:], dpsi[:], AF.Square)
                else:
                    nc.vector._custom_dve(addsq, out=ui[:],
                                          in0=skwi[:], in1=dpsi[:])
                dpsr = psB.tile([128, 1024], F32, tag="pb",
                                name=f"dpsr_{h}_{I}")
                for nh in range(2):
                    ns = slice(nh * 512, (nh + 1) * 512)
                    nc.tensor.matmul(dpsr[:, ns], A[h][:, isl],
                                     Knat[h][:, ns], start=True, stop=False)
                    sk = skwr[:, :, 0, ns] if mode == "a" else skwr[:, :, ns]
                    nc.tensor.matmul(dpsr[:, ns], ident2[:], sk,
                                     start=False, stop=True, perf_mode=DR)
                m2 = pw.tile([128, 1024], F16, tag="m2", bufs=12,
                             name=f"m2_{h}_{I}")
                nc.vector._custom_dve(sqacc, out=m2[:],
                                      in0=ui[:], in1=dpsr[:])
                return m2

            def emit_sqrt(h, I, m2):
                # in-place: mag overwrites m2
                nc.scalar.activation(m2[:], m2[:], AF.Sqrt)
                return m2

            def emit_exp(h, I, mag, rs8, col):
                attn = pw.tile([128, 1024], F16, tag="attn", bufs=10,
                               name=f"attn_{h}_{I}")
                nc.scalar.activation(attn[:], mag[:], AF.Exp,
                                     accum_out=rs8[:, col:col + 1])
                return attn

            def stage_C1(h, I, attn, rc8, col):
                nc.vector.tensor_scalar_mul(attn[:], attn[:],
                                            rc8[:, col:col + 1])
                return attn

            def stage_C2(h, I, attn, atP):
                half = slice((I % 2) * 128, (I % 2) * 128 + 128)
                nc.sync.dma_start(atP[:, :, half], attn[:], transpose=True)

            def stage_D_pair(h, I0, atP):
                # tiles (h, I0) and (h, I0+1) share one AV matmul pass
                isl = slice(I0 * 128, (I0 + 2) * 128)
                avs = psU.tile([128, 256], F32, tag="pu",
                               name=f"avs_{h}_{I0}")
                vsl = slice(h * 128, (h + 1) * 128)
                for J in range(NT):
                    nc.tensor.matmul(avs[:], Vpp[:, J, vsl],
                                     atP[:, J, :],
                                     start=(J == 0), stop=(J == NT - 1))
                nc.vector.tensor_copy(OT[h][:, isl], avs[:])

            def emit_outproj(nh):
                ns = slice(nh * 512, (nh + 1) * 512)
                for part, wo_s in ((0, wo_re), (1, wo_im)):
                    for dt_ in range(4):
                        ds = slice(dt_ * 128, (dt_ + 1) * 128)
                        ops = psU.tile([128, 512], F32, tag="pu",
                                       name=f"ops_{part}_{dt_}_{nh}")
                        for j in range(4):
                            nc.tensor.matmul(ops[:], wo_s[:, j, ds],
                                             OT[j][:, ns],
                                             start=(j == 0), stop=(j == 3))
                        osb = pw.tile([128, 512], F16, tag="osb", bufs=3,
                                      name=f"osb_{part}_{dt_}_{nh}")
                        nc.scalar.copy(osb[:], ops[:])
                        dst = o_r if part == 0 else o_i
                        nc.sync.dma_start(
                            bass.AP(dst, dt_ * 128 * N + nh * 512,
                                    [[N, 128], [1, 512]]),
                            osb[:])

            flat = [(h, I) for h in range(HPC) for I in range(NT)]
            NF = len(flat)
            (qe_map, qe_done, slotmap, skewmap, m2map, magmap, attnmap,
             atPmap) = ({} for _ in range(8))
            rs8map, rc8map = {}, {}
            for s in range(NF + PD + 1):
                if punits:
                    kind, a1, a2 = punits.pop(0)
                    if kind == "v":
                        emit_vproj_unit(a1)
                    else:
                        emit_proj_unit(kind, a1, a2)
                if s < NF:
                    h, I = flat[s]
                    qe_map[(h, I)] = pw.tile([128, 2, WIN], F8, tag="qe",
                                             bufs=6, name=f"qe_{h}_{I}")
                    emit_qrel_part(h, I, 0, qe_map[(h, I)])
                if PW <= s < NF + PW:
                    h, I = flat[s - PW]
                    slotmap[(h, I)] = emit_qrel_write(h, I,
                                                      qe_done.pop((h, I)))
                if PR <= s < NF + PR:
                    h, I = flat[s - PR]
                    skewmap[(h, I)] = emit_qrel_read(
                        h, I, slotmap.pop((h, I)),
                        "a" if (s - PR) % 5 == 2 else "b")
                # batched Sqrt+Exp (SQG tiles), lagged PL iterations;
                # priority-pushed so later iterations' table-neutral copies
                # interleave instead of stalling behind the burst
                t = s - PB - PL
                if 0 <= t < NF and t % SQG == SQG - 1:
                    g = t // SQG
                    prio0 = tc.cur_priority
                    tc.cur_priority = prio0 + PRIO_BUMP
                    for tt in range(t - SQG + 1, t + 1):
                        hh, ii = flat[tt]
                        magmap[(hh, ii)] = emit_sqrt(hh, ii,
                                                     m2map.pop((hh, ii)))
                    rs8 = pw.tile([128, SQG], F32, tag="sm", bufs=3,
                                  name=f"rs8_{g}")
                    rs8map[g] = rs8
                    for tt in range(t - SQG + 1, t + 1):
                        hh, ii = flat[tt]
                        attnmap[(hh, ii)] = emit_exp(hh, ii,
                                                     magmap.pop((hh, ii)),
                                                     rs8, tt % SQG)
                    rc8 = pw.tile([128, SQG], F32, tag="sm", bufs=3,
                                  name=f"rc8_{g}")
                    nc.vector.reciprocal(rc8[:], rs8[:])
                    rc8map[g] = rc8
                    tc.cur_priority = prio0
                if PC - 1 <= s < NF + PC - 1:
                    h, I = flat[s - PC + 1]
                    t1 = s - PC + 1
                    attnmap[(h, I)] = stage_C1(h, I, attnmap.pop((h, I)),
                                               rc8map[t1 // SQG], t1 % SQG)
                if PC <= s < NF + PC:
                    h, I = flat[s - PC]
                    if I % 2 == 0:
                        atPmap[(h, I // 2)] = pw.tile(
                            [128, 8, 256], F16, tag="att", bufs=3,
                            name=f"atP_{h}_{I // 2}")
                    stage_C2(h, I, attnmap.pop((h, I)), atPmap[(h, I // 2)])
                if PD <= s < NF + PD and (s - PD) % 2 == 1:
                    h, I = flat[s - PD]
                    stage_D_pair(h, I - 1, atPmap.pop((h, I // 2)))
                    if (h, I) == (HPC - 1, 3):
                        emit_outproj(0)
                if PB <= s < NF + PB:
                    h, I = flat[s - PB]
                    m2map[(h, I)] = stage_B(h, I, skewmap.pop((h, I)),
                                            "a" if (s - PB) % 5 == 2
                                            else "b")
                if s < NF:
                    h, I = flat[s]
                    qe = qe_map.pop((h, I))
                    emit_qrel_part(h, I, 1, qe)
                    emit_qe_bcast(h, I, qe)
                    qe_done[(h, I)] = qe
            emit_outproj(1)

    nc.compile()
    return nc, mag2


def _prep_core_inputs(inputs, core):
    import ml_dtypes
    b, half = core // 2, core % 2
    x = inputs["x"]
    f16 = np.float16
    xt_r = np.ascontiguousarray(x[b, :, :, 0].T).astype(f16)
    xt_i = np.ascontiguousarray(x[b, :, :, 1].T).astype(f16)

    def pack_ab(wr, wi):
        a = np.empty((DIM, 512), f16)
        bb = np.empty((DIM, 512), f16)
        for hl in range(HPC):
            gh = half * HPC + hl
            cs = slice(gh * DH, (gh + 1) * DH)
            a[:, hl * 128:hl * 128 + 64] = wr[:, cs]
            a[:, hl * 128 + 64:hl * 128 + 128] = wi[:, cs]
            bb[:, hl * 128:hl * 128 + 64] = -wi[:, cs]
            bb[:, hl * 128 + 64:hl * 128 + 128] = wr[:, cs]
        return a, bb

    wq_a, wq_b = pack_ab(inputs["wq_r"], inputs["wq_i"])
    wk_a, wk_b = pack_ab(inputs["wkv_r"][:, :512], inputs["wkv_i"][:, :512])
    wv_a, wv_b = pack_ab(inputs["wkv_r"][:, 512:], inputs["wkv_i"][:, 512:])

    # wo rows permuted head-major: per head hl, rows [r(64); i-part(64)]
    rs0 = half * 256
    wo_re = np.empty((DIM, 512), f16)
    wo_im = np.empty((DIM, 512), f16)
    for hl in range(HPC):
        rr = slice(rs0 + hl * 64, rs0 + (hl + 1) * 64)
        dst_r = slice(hl * 128, hl * 128 + 64)
        dst_i = slice(hl * 128 + 64, hl * 128 + 128)
        wo_re[dst_r] = inputs["wo_r"][rr, :]
        wo_re[dst_i] = -inputs["wo_i"][rr, :]
        wo_im[dst_r] = inputs["wo_i"][rr, :]
        wo_im[dst_i] = inputs["wo_r"][rr, :]

    e = np.arange(2047)
    t_ext = inputs["rel_emb"][np.clip(e - 1023, -MAX_POS, MAX_POS) + MAX_POS]
    relrev = t_ext[::-1].astype(np.float32)      # [2047, 64]
    rel_r = np.zeros((128, 2048), f16)
    rel_i = np.zeros((128, 2048), f16)
    rel_r[0:64, 0:2047] = relrev.T.astype(f16)
    rel_i[64:128, 0:2047] = (-relrev.T).astype(f16)

    smask = np.concatenate(
        [np.full(64, SCALE, np.float32),
         np.full(64, -SCALE, np.float32)]).reshape(128, 1)

    ident2 = np.zeros((64, 2, 128), np.float32)
    for p in range(64):
        for k in range(2):
            ident2[p, k, 2 * p + k] = 1.0
    ident2 = ident2.reshape(64, 256).astype(ml_dtypes.float8_e4m3)

    return {
        "xt_r": xt_r, "xt_i": xt_i,
        "wq_a": wq_a, "wq_b": wq_b, "wk_a": wk_a, "wk_b": wk_b,
        "wv_a": wv_a, "wv_b": wv_b, "wo_re": wo_re, "wo_im": wo_im,
        "rel_r": rel_r, "rel_i": rel_i, "smask": smask,
        "ident2": ident2,
    }


_last_results = {}


def kernel(**inputs):
    inputs = {k: np.asarray(v) for k, v in inputs.items()}
    nc, _ = build_module()
    in_maps = [_prep_core_inputs(inputs, c) for c in range(8)]
    res = run_bass_kernel_spmd(nc, in_maps, core_ids=list(range(8)))
    _last_results["res"] = res

    bo_r = inputs["bo_r"].astype(np.float32)
    bo_i = inputs["bo_i"].astype(np.float32)
    out = np.empty((B, N, DIM, 2), np.float32)
    for b in range(B):
        r = (res.results[2 * b]["o_r"].astype(np.float32)
             + res.results[2 * b + 1]["o_r"].astype(np.float32))
        i = (res.results[2 * b]["o_i"].astype(np.float32)
             + res.results[2 * b + 1]["o_i"].astype(np.float32))
        out[b, :, :, 0] = r.T + bo_r[None, :]
        out[b, :, :, 1] = i.T + bo_i[None, :]
    return out


# revision 65
# speedup vs baseline: 1.0462x; 1.0306x over previous
"""Complex-valued relative-position attention (nn_CAttention) on 8 TRN2 cores.

Sharding: batch (4) x head-half (2) -> 8 cores. Each core computes its
batch's projections for its 4 heads, full attention for those heads, and a
row-split partial output projection. Host sums the two partial outputs per
batch, adds the output bias, and restacks.

Design (v9, ~221.7us vs v3's 239.5us):
  - Skew-add on the PE: the qrel skew values are accumulated into the dots
    PSUM by fp8 DoubleRow identity matmuls; the readback DMA lands the
    diagonal band directly in [64, ko=2, 1024] DoubleRow layout (part r).
    Most tiles (mode "b") read part i in natural [128,1024] layout and fold
    it via one ADDSQ; every ~5th tile (mode "a") instead identity-adds part
    i in PSUM and squares it on ACT, balancing ACT vs DVE. A single SQACC
    (ei + dpsr^2, one PSUM operand each - the DVE PSUM-port limit) yields
    m2.
  - rel clip regions: columns e<=511 / e>=1535 of the reversed rel table
    are constant and ADJOIN each tile's varying span in e-space, so the
    two const columns ride inside the chunk matmuls (no extra mini-matmul
    or PSUM slot); GPSIMD broadcasts them across the clip regions of the
    fp8 qe tile before the slot write (28% less qrel staging on ACT/DVE,
    22% fewer qrel matmul columns, 2 fewer psU allocations per tile).
  - mode "a" tiles sit at phase 3 of each SQG=8 batch (t%8==3), filling
    the ACT gap mid-batch; the natural-layout part-i skew reads go on the
    sync DMA queue (scalar-queue DMA waits would block activations).
  - dots_i uses a second stationary A_i = [qi*s; qr*s] derived from A by
    two small SBUF copies, replacing the Kni2 staging copies.
  - Vpp is one [128, 8, 512] tile (one copy per v unit); AV output lands
    in per-head OT tiles with a single [128,256] copy (wo is host-permuted
    to head-major rows to match).
  - Softmax: Sqrt in-place on m2, Exp (ACT, batched per 8 tiles for table
    amortization); rowsums accumulate into a shared [128,8] tile, one
    batched reciprocal per group; attn scaling runs on DVE (4x mode).
"""
import functools
import numpy as np

import concourse.bass as bass
import concourse.bacc as bacc
import concourse.mybir as mybir
import concourse.tile as tile
from concourse.bass_utils import run_bass_kernel_spmd

F32 = mybir.dt.float32
F16 = mybir.dt.float16
F8 = mybir.dt.float8e4
AF = mybir.ActivationFunctionType
DR = mybir.MatmulPerfMode.DoubleRow

HEADS, DH, MAX_POS = 8, 64, 512
B, N, DIM = 4, 1024, 512
HPC = 4            # heads per core
KT = 4             # dim k-tiles (512/128)
NT = 8             # n tiles (1024/128)
WIN = 1152         # qrel window width (>= 1151)
SCALE = DH ** (-0.5)
PW = 1             # slot write offset (copies at s, write at s+PW)
PR = 2             # skew readback offset
PB = 5             # stage B offset (skew round-trip prefetch distance)
PL = 2             # batch lag beyond PB (tiles fully ready -> no table leak)
PC = 16            # stage C offset (attn ready after batched Exp)
PD = 17            # stage D offset (processes tile PAIRS on odd steps)
SQG = 8            # sqrt/exp table-batching group size
PRIO_BUMP = 250    # batch priority push


def register_mag2():
    from concourse import dve_ops
    from concourse.dve_spec import Spec, Src0, Src1, AluOp, Bin, lower, sq
    from concourse.dve_uop import DveOpSpec

    existing = [op for op in dve_ops.OPS
                if op.name in ("MAG2_ANT", "ADDSQ_ANT", "SQACC_ANT")]
    if len(existing) == 3:
        return existing

    def reg(name, body, ref):
        spec = Spec(body=body, reference=ref)
        opcode = dve_ops._CUSTOM_DVE_ROW_BASE + len(dve_ops.OPS)
        shas = {}
        for ver in ("v3",):
            s = DveOpSpec(name=name, opcode=opcode,
                          uops=lower(spec, ver=ver), rd1_en=True)
            shas[ver] = s.sha(ver)
        op = dve_ops.DveOp(name, spec, subdim=False, uops_sha=shas)
        dve_ops._SUB_OPCODE_FOR_NAME[op.name] = opcode
        dve_ops.OPS.append(op)
        dve_ops.CUSTOM_DVE_SPECS[op.name] = op.spec
        return op

    op1 = reg("MAG2_ANT", Bin(AluOp.ADD, sq(Src0), sq(Src1)),
              lambda in0, in1, s0, s1, imm2: (
                  in0.astype(np.float32) ** 2 + in1.astype(np.float32) ** 2))
    op2 = reg("ADDSQ_ANT", sq(Bin(AluOp.ADD, Src0, Src1)),
              lambda in0, in1, s0, s1, imm2: (
                  (in0.astype(np.float32) + in1.astype(np.float32)) ** 2))
    op3 = reg("SQACC_ANT", Bin(AluOp.ADD, Src0, sq(Src1)),
              lambda in0, in1, s0, s1, imm2: (
                  in0.astype(np.float32) + in1.astype(np.float32) ** 2))
    return op1, op2, op3


def c_lo(i_blk):
    return 896 - 128 * i_blk


@functools.cache
def build_module():
    import concourse.tile_utils as tile_utils
    if getattr(tile_utils, "max_sbuf_usage", 0) < 208 * 1024:
        tile_utils.max_sbuf_usage = 208 * 1024

    mag2, addsq, sqacc = register_mag2()
    nc = bacc.Bacc("TRN2", target_bir_lowering=False, debug=False,
                   num_devices=8, dynamic_dma_scratch_size=16384)

    din = {}
    for nm, shape, dt_ in [
        ("xt_r", [DIM, N], F16), ("xt_i", [DIM, N], F16),
        ("wq_a", [DIM, 512], F16), ("wq_b", [DIM, 512], F16),
        ("wk_a", [DIM, 512], F16), ("wk_b", [DIM, 512], F16),
        ("wv_a", [DIM, 512], F16), ("wv_b", [DIM, 512], F16),
        ("wo_re", [DIM, 512], F16), ("wo_im", [DIM, 512], F16),
        ("rel_r", [128, 2048], F16), ("rel_i", [128, 2048], F16),
        ("smask", [128, 1], F32),
        ("ident2", [64, 256], F8),
    ]:
        din[nm] = nc.dram_tensor(nm, shape, dt_, kind="ExternalInput")
    o_r = nc.dram_tensor("o_r", [DIM, N], F16, kind="ExternalOutput")
    o_i = nc.dram_tensor("o_i", [DIM, N], F16, kind="ExternalOutput")

    with tile.TileContext(nc) as tc:
        with (
            tc.tile_pool(name="const", bufs=1) as cpool,
            tc.tile_pool(name="work", bufs=2) as pw,
            tc.tile_pool(name="psB", bufs=2, space="PSUM") as psB,
            tc.tile_pool(name="psU", bufs=4, space="PSUM") as psU,
            tc.tile_pool(name="dram", bufs=16, space="DRAM") as pdram,
        ):
            # ---------------- constants ----------------
            hengs = (nc.sync, nc.scalar)
            smask = cpool.tile([128, 1], F32, tag="smask")
            nc.sync.dma_start(smask[:], din["smask"][:, :])
            ident2 = cpool.tile([64, 2, 128], F8, tag="ident2")
            nc.scalar.dma_start(
                ident2[:], bass.AP(din["ident2"], 0,
                                   [[256, 64], [128, 2], [1, 128]]))

            # load order tuned so Q(0)'s inputs land first
            xtt = {}
            qd = 0

            def load_xt(nm):
                nonlocal qd
                t = pw.tile([128, 4, 1024], F16, tag="xt", bufs=2, name=nm)
                hengs[qd % 2].dma_start(
                    t[:], bass.AP(din[nm], 0,
                                  [[N, 128], [128 * N, 4], [1, N]]))
                qd += 1
                xtt[nm] = t

            def xt(nm, kt, nh):
                return xtt[nm][:, kt, nh * 512:(nh + 1) * 512]

            def load_w(nm, tag, bufs):
                # one [128, 4, 512] tile per weight tensor, single DMA
                nonlocal qd
                t = pw.tile([128, 4, 512], F16, tag=tag, bufs=bufs,
                            name=nm)
                hengs[qd % 2].dma_start(
                    t[:], bass.AP(din[nm], 0,
                                  [[512, 128], [128 * 512, 4], [1, 512]]))
                qd += 1
                return [t[:, kt, :] for kt in range(KT)]

            wqa = load_w("wq_a", "wl", 4)
            load_xt("xt_r")
            wqb = load_w("wq_b", "wl", 4)
            load_xt("xt_i")
            rel_r = cpool.tile([128, 2048], F16, tag="rel_r")
            nc.sync.dma_start(rel_r[:], din["rel_r"][:, :])
            wka = load_w("wk_a", "wl", 4)
            wkb = load_w("wk_b", "wl", 4)
            rel_i = cpool.tile([128, 2048], F16, tag="rel_i")
            nc.scalar.dma_start(rel_i[:], din["rel_i"][:, :])
            wva = load_w("wv_a", "wv", 2)
            wvb = load_w("wv_b", "wv", 2)
            wo_re = cpool.tile([128, 4, 512], F16, tag="wo_re")
            wo_im = cpool.tile([128, 4, 512], F16, tag="wo_im")
            nc.sync.dma_start(
                wo_re[:], bass.AP(din["wo_re"], 0,
                                  [[512, 128], [128 * 512, 4], [1, 512]]))
            nc.scalar.dma_start(
                wo_im[:], bass.AP(din["wo_im"], 0,
                                  [[512, 128], [128 * 512, 4], [1, 512]]))

            A = [None] * HPC
            Ai = [None] * HPC
            Knat = [None] * HPC
            Vpp = pw.tile([128, 8, 512], F16, tag="vpp", bufs=1,
                          name="Vpp")

            def emit_proj_unit(kind, h, nh):
                wa, wb = (wqa, wqb) if kind == "q" else (wka, wkb)
                hs = slice(h * 128, (h + 1) * 128)
                ns = slice(nh * 512, (nh + 1) * 512)
                if kind == "q" and A[h] is None:
                    A[h] = pw.tile([128, 1024], F16, tag="stk", bufs=12,
                                   name=f"A{h}")
                    Ai[h] = pw.tile([128, 1024], F16, tag="stk", bufs=12,
                                    name=f"Ai{h}")
                if kind == "k" and Knat[h] is None:
                    Knat[h] = pw.tile([128, 1024], F16, tag="stk",
                                      bufs=12, name=f"Knat{h}")
                ps = psU.tile([128, 512], F32, tag="pu",
                              name=f"ps{kind}_{h}_{nh}")
                for kt in range(KT):
                    nc.tensor.matmul(ps[:], wa[kt][:, hs],
                                     xt("xt_r", kt, nh),
                                     start=(kt == 0), stop=False)
                for kt in range(KT):
                    nc.tensor.matmul(ps[:], wb[kt][:, hs],
                                     xt("xt_i", kt, nh),
                                     start=False, stop=(kt == KT - 1))
                if kind == "q":
                    nc.vector.tensor_scalar_mul(A[h][:, ns], ps[:],
                                                smask[:])
                    # A_i = [qi*s; qr*s] from A = [qr*s; -qi*s]
                    nc.vector.tensor_scalar_mul(Ai[h][0:64, ns],
                                                A[h][64:128, ns], -1.0)
                    nc.vector.tensor_copy(Ai[h][64:128, ns],
                                          A[h][0:64, ns])
                else:
                    nc.scalar.copy(Knat[h][:, ns], ps[:])

            def emit_vproj_unit(J):
                xs = slice((J % 4) * 128, (J % 4) * 128 + 128)
                vps = psU.tile([128, 512], F32, tag="pu", name=f"vps_{J}")
                for kt in range(KT):
                    nc.tensor.matmul(vps[:],
                                     xt("xt_r", kt, J // 4)[:, xs],
                                     wva[kt][:, :],
                                     start=(kt == 0), stop=False)
                for kt in range(KT):
                    nc.tensor.matmul(vps[:],
                                     xt("xt_i", kt, J // 4)[:, xs],
                                     wvb[kt][:, :],
                                     start=False, stop=(kt == KT - 1))
                nc.vector.tensor_copy(Vpp[:, J, :], vps[:])

            # head 0 Q/K up front; the rest feeds the loop's PE slack
            for kind in ("q", "k"):
                for nh in range(2):
                    emit_proj_unit(kind, 0, nh)
            punits = [("q", 1, 0), ("q", 1, 1), ("k", 1, 0), ("k", 1, 1),
                      ("v", 0, None), ("v", 1, None), ("v", 2, None),
                      ("v", 3, None),
                      ("q", 2, 0), ("q", 2, 1), ("k", 2, 0), ("k", 2, 1),
                      ("v", 4, None), ("v", 5, None), ("v", 6, None),
                      ("v", 7, None),
                      ("q", 3, 0), ("q", 3, 1), ("k", 3, 0), ("k", 3, 1)]

            # OT stacks: per-head [avr(64); avi(64)] x n, [128, 1024] fp16
            OT = [pw.tile([128, 1024], F16, tag="otk", bufs=4,
                          name=f"OT{t}") for t in range(4)]

            # ---------------- attention pipeline stages ----------------
            # rel columns e in [512, 1534] vary; e<=511 are all rel_emb[2M]
            # and e>=1535 all rel_emb[0]. Only the varying span + 2 const
            # columns are computed/staged; Pool broadcasts the const
            # regions into qe before the slot write.
            def qwin(I):
                lo = c_lo(I)
                head = max(0, 512 - lo)
                tail = max(0, lo + WIN - 1535)
                return head, WIN - head - tail

            def emit_qrel_part(h, I, part, qe):
                # The const columns (e=511 head / e=1535 tail) adjoin the
                # varying span in e-space, so they ride inside the chunk
                # matmuls; the broadcasts later replicate them in qe.
                isl = slice(I * 128, (I + 1) * 128)
                lo = c_lo(I)
                head, w = qwin(I)
                tail = WIN - head - w
                e0 = lo + head - (1 if head else 0)
                W = w + (1 if head else 0) + (1 if tail else 0)
                q0 = head - (1 if head else 0)
                relt = rel_r if part == 0 else rel_i
                chunks = []
                c = 0
                while c < W:
                    chunks.append((c, min(c + 512, W)))
                    c += 512
                qpss = []
                for ci, (c0, c1) in enumerate(chunks):
                    qps = psU.tile([128, c1 - c0], F32, tag="pu",
                                   name=f"qps{part}_{h}_{I}_{ci}")
                    nc.tensor.matmul(qps[:], A[h][:, isl],
                                     relt[:, e0 + c0:e0 + c1],
                                     start=True, stop=True)
                    qpss.append(qps)
                for ci, (c0, c1) in enumerate(chunks):
                    dst = qe[:, part, q0 + c0:q0 + c1]
                    if ci == 0:
                        nc.vector.tensor_copy(dst, qpss[ci][:])
                    else:
                        nc.scalar.copy(dst, qpss[ci][:])

            def emit_qe_bcast(h, I, qe):
                head, w = qwin(I)
                for part in range(2):
                    if head > 1:
                        nc.gpsimd.tensor_copy(
                            qe[:, part, 0:head - 1],
                            qe[:, part, head - 1:head].broadcast_to(
                                [128, head - 1]))
                    if head + w + 1 < WIN:
                        nc.gpsimd.tensor_copy(
                            qe[:, part, head + w + 1:WIN],
                            qe[:, part, head + w:head + w + 1].broadcast_to(
                                [128, WIN - head - w - 1]))

            def emit_qrel_write(h, I, qe):
                slot = pdram.tile([128, 2 * WIN], F8, tag="qrev",
                                  name=f"qrev_{h}_{I}")
                nc.gpsimd.dma_start(
                    bass.AP(slot.tensor, 0,
                            [[2 * WIN, 128], [WIN, 2], [1, WIN]]),
                    qe[:])
                return slot

            def emit_qrel_read(h, I, slot, mode):
                # part r in DoubleRow layout: (p, ko, j) <- slot row
                # 2p+ko, band col 127-(2p+ko)+j
                if mode == "a":
                    skw = pw.tile([64, 2, 2, 1024], F8, tag="skw", bufs=2,
                                  name=f"skew_{h}_{I}")
                    nc.sync.dma_start(
                        skw[:],
                        bass.AP(slot.tensor, 127,
                                [[2 * (2 * WIN - 1), 64], [2 * WIN - 1, 2],
                                 [WIN, 2], [1, 1024]]))
                    return skw, None
                skwr = pw.tile([64, 2, 1024], F8, tag="skwr", bufs=6,
                               name=f"skewr_{h}_{I}")
                nc.sync.dma_start(
                    skwr[:],
                    bass.AP(slot.tensor, 127,
                            [[2 * (2 * WIN - 1), 64], [2 * WIN - 1, 2],
                             [1, 1024]]))
                # part i natural: (p, j) <- slot row p, col W+127-p+j
                skwi = pw.tile([128, 1024], F8, tag="skwi", bufs=6,
                               name=f"skewi_{h}_{I}")
                nc.sync.dma_start(
                    skwi[:],
                    bass.AP(slot.tensor, WIN + 127,
                            [[2 * WIN - 1, 128], [1, 1024]]))
                return skwr, skwi

            def stage_B(h, I, skws, mode):
                skwr, skwi = skws
                isl = slice(I * 128, (I + 1) * 128)
                # i part first: its SBUF crossing overlaps the r matmuls
                dpsi = psB.tile([128, 1024], F32, tag="pb",
                                name=f"dpsi_{h}_{I}")
                for nh in range(2):
                    ns = slice(nh * 512, (nh + 1) * 512)
                    nc.tensor.matmul(dpsi[:, ns], Ai[h][:, isl],
                                     Knat[h][:, ns], start=True,
                                     stop=(mode == "b"))
                    if mode == "a":
                        nc.tensor.matmul(dpsi[:, ns], ident2[:],
                                         skwr[:, :, 1, ns], start=False,
                                         stop=True, perf_mode=DR)
                ui = pw.tile([128, 1024], F16, tag="ui", bufs=3,
                             name=f"ui_{h}_{I}")
                if mode == "a":
                    nc.scalar.activation(ui[:], dpsi[:], AF.Square)
                else:
                    nc.vector._custom_dve(addsq, out=ui[:],
                                          in0=skwi[:], in1=dpsi[:])
                dpsr = psB.tile([128, 1024], F32, tag="pb",
                                name=f"dpsr_{h}_{I}")
                for nh in range(2):
                    ns = slice(nh * 512, (nh + 1) * 512)
                    nc.tensor.matmul(dpsr[:, ns], A[h][:, isl],
                                     Knat[h][:, ns], start=True, stop=False)
                    sk = skwr[:, :, 0, ns] if mode == "a" else skwr[:, :, ns]
                    nc.tensor.matmul(dpsr[:, ns], ident2[:], sk,
                                     start=False, stop=True, perf_mode=DR)
                m2 = pw.tile([128, 1024], F16, tag="m2", bufs=12,
                             name=f"m2_{h}_{I}")
                nc.vector._custom_dve(sqacc, out=m2[:],
                                      in0=ui[:], in1=dpsr[:])
                return m2

            def emit_sqrt(h, I, m2):
                # in-place: mag overwrites m2
                nc.scalar.activation(m2[:], m2[:], AF.Sqrt)
                return m2

            def emit_exp(h, I, mag, rs8, col):
                attn = pw.tile([128, 1024], F16, tag="attn", bufs=10,
                               name=f"attn_{h}_{I}")
                nc.scalar.activation(attn[:], mag[:], AF.Exp,
                                     accum_out=rs8[:, col:col + 1])
                return attn

            def stage_C1(h, I, attn, rc8, col):
                nc.vector.tensor_scalar_mul(attn[:], attn[:],
                                            rc8[:, col:col + 1])
                return attn

            def stage_C2(h, I, attn, atP):
                half = slice((I % 2) * 128, (I % 2) * 128 + 128)
                nc.sync.dma_start(atP[:, :, half], attn[:], transpose=True)

            def stage_D_pair(h, I0, atP):
                # tiles (h, I0) and (h, I0+1) share one AV matmul pass
                isl = slice(I0 * 128, (I0 + 2) * 128)
                avs = psU.tile([128, 256], F32, tag="pu",
                               name=f"avs_{h}_{I0}")
                vsl = slice(h * 128, (h + 1) * 128)
                for J in range(NT):
                    nc.tensor.matmul(avs[:], Vpp[:, J, vsl],
                                     atP[:, J, :],
                                     start=(J == 0), stop=(J == NT - 1))
                nc.vector.tensor_copy(OT[h][:, isl], avs[:])

            def emit_outproj(nh):
                ns = slice(nh * 512, (nh + 1) * 512)
                for part, wo_s in ((0, wo_re), (1, wo_im)):
                    for dt_ in range(4):
                        ds = slice(dt_ * 128, (dt_ + 1) * 128)
                        ops = psU.tile([128, 512], F32, tag="pu",
                                       name=f"ops_{part}_{dt_}_{nh}")
                        for j in range(4):
                            nc.tensor.matmul(ops[:], wo_s[:, j, ds],
                                             OT[j][:, ns],
                                             start=(j == 0), stop=(j == 3))
                        osb = pw.tile([128, 512], F16, tag="osb", bufs=3,
                                      name=f"osb_{part}_{dt_}_{nh}")
                        nc.scalar.copy(osb[:], ops[:])
                        dst = o_r if part == 0 else o_i
                        nc.sync.dma_start(
                            bass.AP(dst, dt_ * 128 * N + nh * 512,
                                    [[N, 128], [1, 512]]),
                            osb[:])

            flat = [(h, I) for h in range(HPC) for I in range(NT)]
            NF = len(flat)
            (qe_map, qe_done, slotmap, skewmap, m2map, magmap, attnmap,
             atPmap) = ({} for _ in range(8))
            rs8map, rc8map = {}, {}
            for s in range(NF + PD + 1):
                if punits:
                    kind, a1, a2 = punits.pop(0)
                    if kind == "v":
                        emit_vproj_unit(a1)
                    else:
                        emit_proj_unit(kind, a1, a2)
                if s < NF:
                    h, I = flat[s]
                    qe_map[(h, I)] = pw.tile([128, 2, WIN], F8, tag="qe",
                                             bufs=6, name=f"qe_{h}_{I}")
                    emit_qrel_part(h, I, 0, qe_map[(h, I)])
                if PW <= s < NF + PW:
                    h, I = flat[s - PW]
                    slotmap[(h, I)] = emit_qrel_write(h, I,
                                                      qe_done.pop((h, I)))
                if PR <= s < NF + PR:
                    h, I = flat[s - PR]
                    skewmap[(h, I)] = emit_qrel_read(
                        h, I, slotmap.pop((h, I)),
                        "a" if (s - PR) % 8 == 3 else "b")
                # batched Sqrt+Exp (SQG tiles), lagged PL iterations;
                # priority-pushed so later iterations' table-neutral copies
                # interleave instead of stalling behind the burst
                t = s - PB - PL
                if 0 <= t < NF and t % SQG == SQG - 1:
                    g = t // SQG
                    prio0 = tc.cur_priority
                    tc.cur_priority = prio0 + PRIO_BUMP
                    for tt in range(t - SQG + 1, t + 1):
                        hh, ii = flat[tt]
                        magmap[(hh, ii)] = emit_sqrt(hh, ii,
                                                     m2map.pop((hh, ii)))
                    rs8 = pw.tile([128, SQG], F32, tag="sm", bufs=3,
                                  name=f"rs8_{g}")
                    rs8map[g] = rs8
                    for tt in range(t - SQG + 1, t + 1):
                        hh, ii = flat[tt]
                        attnmap[(hh, ii)] = emit_exp(hh, ii,
                                                     magmap.pop((hh, ii)),
                                                     rs8, tt % SQG)
                    rc8 = pw.tile([128, SQG], F32, tag="sm", bufs=3,
                                  name=f"rc8_{g}")
                    nc.vector.reciprocal(rc8[:], rs8[:])
                    rc8map[g] = rc8
                    tc.cur_priority = prio0
                if PC - 1 <= s < NF + PC - 1:
                    h, I = flat[s - PC + 1]
                    t1 = s - PC + 1
                    attnmap[(h, I)] = stage_C1(h, I, attnmap.pop((h, I)),
                                               rc8map[t1 // SQG], t1 % SQG)
                if PC <= s < NF + PC:
                    h, I = flat[s - PC]
                    if I % 2 == 0:
                        atPmap[(h, I // 2)] = pw.tile(
                            [128, 8, 256], F16, tag="att", bufs=3,
                            name=f"atP_{h}_{I // 2}")
                    stage_C2(h, I, attnmap.pop((h, I)), atPmap[(h, I // 2)])
                if PD <= s < NF + PD and (s - PD) % 2 == 1:
                    h, I = flat[s - PD]
                    stage_D_pair(h, I - 1, atPmap.pop((h, I // 2)))
                    if (h, I) == (HPC - 1, 3):
                        emit_outproj(0)
                if PB <= s < NF + PB:
                    h, I = flat[s - PB]
                    m2map[(h, I)] = stage_B(h, I, skewmap.pop((h, I)),
                                            "a" if (s - PB) % 8 == 3
                                            else "b")
                if s < NF:
                    h, I = flat[s]
                    qe = qe_map.pop((h, I))
                    emit_qrel_part(h, I, 1, qe)
                    emit_qe_bcast(h, I, qe)
                    qe_done[(h, I)] = qe
            emit_outproj(1)

    nc.compile()
    return nc, mag2


def _prep_core_inputs(inputs, core):
    import ml_dtypes
    b, half = core // 2, core % 2
    x = inputs["x"]
    f16 = np.float16
    xt_r = np.ascontiguousarray(x[b, :, :, 0].T).astype(f16)
    xt_i = np.ascontiguousarray(x[b, :, :, 1].T).astype(f16)

    def pack_ab(wr, wi):
        a = np.empty((DIM, 512), f16)
        bb = np.empty((DIM, 512), f16)
        for hl in range(HPC):
            gh = half * HPC + hl
            cs = slice(gh * DH, (gh + 1) * DH)
            a[:, hl * 128:hl * 128 + 64] = wr[:, cs]
            a[:, hl * 128 + 64:hl * 128 + 128] = wi[:, cs]
            bb[:, hl * 128:hl * 128 + 64] = -wi[:, cs]
            bb[:, hl * 128 + 64:hl * 128 + 128] = wr[:, cs]
        return a, bb

    wq_a, wq_b = pack_ab(inputs["wq_r"], inputs["wq_i"])
    wk_a, wk_b = pack_ab(inputs["wkv_r"][:, :512], inputs["wkv_i"][:, :512])
    wv_a, wv_b = pack_ab(inputs["wkv_r"][:, 512:], inputs["wkv_i"][:, 512:])

    # wo rows permuted head-major: per head hl, rows [r(64); i-part(64)]
    rs0 = half * 256
    wo_re = np.empty((DIM, 512), f16)
    wo_im = np.empty((DIM, 512), f16)
    for hl in range(HPC):
        rr = slice(rs0 + hl * 64, rs0 + (hl + 1) * 64)
        dst_r = slice(hl * 128, hl * 128 + 64)
        dst_i = slice(hl * 128 + 64, hl * 128 + 128)
        wo_re[dst_r] = inputs["wo_r"][rr, :]
        wo_re[dst_i] = -inputs["wo_i"][rr, :]
        wo_im[dst_r] = inputs["wo_i"][rr, :]
        wo_im[dst_i] = inputs["wo_r"][rr, :]

    e = np.arange(2047)
    t_ext = inputs["rel_emb"][np.clip(e - 1023, -MAX_POS, MAX_POS) + MAX_POS]
    relrev = t_ext[::-1].astype(np.float32)      # [2047, 64]
    rel_r = np.zeros((128, 2048), f16)
    rel_i = np.zeros((128, 2048), f16)
    rel_r[0:64, 0:2047] = relrev.T.astype(f16)
    rel_i[64:128, 0:2047] = (-relrev.T).astype(f16)

    smask = np.concatenate(
        [np.full(64, SCALE, np.float32),
         np.full(64, -SCALE, np.float32)]).reshape(128, 1)

    ident2 = np.zeros((64, 2, 128), np.float32)
    for p in range(64):
        for k in range(2):
            ident2[p, k, 2 * p + k] = 1.0
    ident2 = ident2.reshape(64, 256).astype(ml_dtypes.float8_e4m3)

    return {
        "xt_r": xt_r, "xt_i": xt_i,
        "wq_a": wq_a, "wq_b": wq_b, "wk_a": wk_a, "wk_b": wk_b,
        "wv_a": wv_a, "wv_b": wv_b, "wo_re": wo_re, "wo_im": wo_im,
        "rel_r": rel_r, "rel_i": rel_i, "smask": smask,
        "ident2": ident2,
    }


_last_results = {}


def kernel(**inputs):
    inputs = {k: np.asarray(v) for k, v in inputs.items()}
    nc, _ = build_module()
    in_maps = [_prep_core_inputs(inputs, c) for c in range(8)]
    res = run_bass_kernel_spmd(nc, in_maps, core_ids=list(range(8)))
    _last_results["res"] = res

    bo_r = inputs["bo_r"].astype(np.float32)
    bo_i = inputs["bo_i"].astype(np.float32)
    out = np.empty((B, N, DIM, 2), np.float32)
    for b in range(B):
        r = (res.results[2 * b]["o_r"].astype(np.float32)
             + res.results[2 * b + 1]["o_r"].astype(np.float32))
        i = (res.results[2 * b]["o_i"].astype(np.float32)
             + res.results[2 * b + 1]["o_i"].astype(np.float32))
        out[b, :, :, 0] = r.T + bo_r[None, :]
        out[b, :, :, 1] = i.T + bo_i[None, :]
    return out


# revision 77
# speedup vs baseline: 1.0582x; 1.0115x over previous
"""Complex-valued relative-position attention (nn_CAttention) on 8 TRN2 cores.

Sharding: batch (4) x head-half (2) -> 8 cores. Each core computes its
batch's projections for its 4 heads, full attention for those heads, and a
row-split partial output projection. Host sums the two partial outputs per
batch, adds the output bias, and restacks.

Design (v11, ~219.2us vs v3's 239.5us):
  - Skew-add on the PE: the qrel skew values are accumulated into the dots
    PSUM by fp8 DoubleRow identity matmuls; the readback DMA lands the
    diagonal band directly in [64, ko=2, 1024] DoubleRow layout (part r).
    Most tiles (mode "b") read part i in natural [128,1024] layout and fold
    it via one ADDSQ; mode "a" tiles instead identity-add part i in PSUM
    and square it on ACT, balancing ACT vs DVE. A single SQACC
    (ei + dpsr^2, one PSUM operand each - the DVE PSUM-port limit) yields
    m2.
  - rel clip regions: columns e<=511 / e>=1535 of the reversed rel table
    are constant and ADJOIN each tile's varying span in e-space, so the
    two const columns ride inside the chunk matmuls (no extra mini-matmul
    or PSUM slot); GPSIMD broadcasts them across the clip regions of the
    fp8 qe tile before the slot write (28% less qrel staging on ACT/DVE,
    22% fewer qrel matmul columns, 2 fewer psU allocations per tile).
  - mode "a" tiles sit at phases 1,3,5 of each SQG=8 batch, filling the
    ACT gap mid-batch; the natural-layout part-i skew reads go on the
    sync DMA queue (scalar-queue DMA waits would block activations).
  - each sqrt/exp batch is chained with no_sync dependencies so the list
    scheduler keeps it adjacent in the ACT queue: exactly 2 table loads
    per batch (9 total instead of 27).
  - dots_i uses a second stationary A_i = [qi*s; qr*s] derived from A by
    two small SBUF copies, replacing the Kni2 staging copies.
  - Vpp is one [128, 8, 512] tile (one copy per v unit); AV output lands
    in per-head OT tiles with a single [128,256] copy (wo is host-permuted
    to head-major rows to match).
  - Softmax: Sqrt in-place on m2, Exp (ACT, batched per 8 tiles for table
    amortization); rowsums accumulate into a shared [128,8] tile, one
    batched reciprocal per group; attn scaling runs on DVE (4x mode).
"""
import functools
import numpy as np

import concourse.bass as bass
import concourse.bacc as bacc
import concourse.mybir as mybir
import concourse.tile as tile
from concourse.bass_utils import run_bass_kernel_spmd

F32 = mybir.dt.float32
F16 = mybir.dt.float16
F8 = mybir.dt.float8e4
AF = mybir.ActivationFunctionType
DR = mybir.MatmulPerfMode.DoubleRow

HEADS, DH, MAX_POS = 8, 64, 512
B, N, DIM = 4, 1024, 512
HPC = 4            # heads per core
KT = 4             # dim k-tiles (512/128)
NT = 8             # n tiles (1024/128)
WIN = 1152         # qrel window width (>= 1151)
SCALE = DH ** (-0.5)
PW = 1             # slot write offset (copies at s, write at s+PW)
PR = 2             # skew readback offset
PB = 5             # stage B offset (skew round-trip prefetch distance)
PL = 2             # batch lag beyond PB (tiles fully ready -> no table leak)
PC = 16            # stage C offset (attn ready after batched Exp)
PD = 17            # stage D offset (processes tile PAIRS on odd steps)
SQG = 8            # sqrt/exp table-batching group size
PRIO_BUMP = 250    # batch priority push


def register_mag2():
    from concourse import dve_ops
    from concourse.dve_spec import Spec, Src0, Src1, AluOp, Bin, lower, sq
    from concourse.dve_uop import DveOpSpec

    existing = [op for op in dve_ops.OPS
                if op.name in ("MAG2_ANT", "ADDSQ_ANT", "SQACC_ANT")]
    if len(existing) == 3:
        return existing

    def reg(name, body, ref):
        spec = Spec(body=body, reference=ref)
        opcode = dve_ops._CUSTOM_DVE_ROW_BASE + len(dve_ops.OPS)
        shas = {}
        for ver in ("v3",):
            s = DveOpSpec(name=name, opcode=opcode,
                          uops=lower(spec, ver=ver), rd1_en=True)
            shas[ver] = s.sha(ver)
        op = dve_ops.DveOp(name, spec, subdim=False, uops_sha=shas)
        dve_ops._SUB_OPCODE_FOR_NAME[op.name] = opcode
        dve_ops.OPS.append(op)
        dve_ops.CUSTOM_DVE_SPECS[op.name] = op.spec
        return op

    op1 = reg("MAG2_ANT", Bin(AluOp.ADD, sq(Src0), sq(Src1)),
              lambda in0, in1, s0, s1, imm2: (
                  in0.astype(np.float32) ** 2 + in1.astype(np.float32) ** 2))
    op2 = reg("ADDSQ_ANT", sq(Bin(AluOp.ADD, Src0, Src1)),
              lambda in0, in1, s0, s1, imm2: (
                  (in0.astype(np.float32) + in1.astype(np.float32)) ** 2))
    op3 = reg("SQACC_ANT", Bin(AluOp.ADD, Src0, sq(Src1)),
              lambda in0, in1, s0, s1, imm2: (
                  in0.astype(np.float32) + in1.astype(np.float32) ** 2))
    return op1, op2, op3


def c_lo(i_blk):
    return 896 - 128 * i_blk


@functools.cache
def build_module():
    import concourse.tile_utils as tile_utils
    if getattr(tile_utils, "max_sbuf_usage", 0) < 208 * 1024:
        tile_utils.max_sbuf_usage = 208 * 1024

    mag2, addsq, sqacc = register_mag2()
    nc = bacc.Bacc("TRN2", target_bir_lowering=False, debug=False,
                   num_devices=8, dynamic_dma_scratch_size=16384)

    din = {}
    for nm, shape, dt_ in [
        ("xt_r", [DIM, N], F16), ("xt_i", [DIM, N], F16),
        ("wq_a", [DIM, 512], F16), ("wq_b", [DIM, 512], F16),
        ("wk_a", [DIM, 512], F16), ("wk_b", [DIM, 512], F16),
        ("wv_a", [DIM, 512], F16), ("wv_b", [DIM, 512], F16),
        ("wo_re", [DIM, 512], F16), ("wo_im", [DIM, 512], F16),
        ("rel_r", [128, 2048], F16), ("rel_i", [128, 2048], F16),
        ("smask", [128, 1], F32),
        ("ident2", [64, 256], F8),
    ]:
        din[nm] = nc.dram_tensor(nm, shape, dt_, kind="ExternalInput")
    o_r = nc.dram_tensor("o_r", [DIM, N], F16, kind="ExternalOutput")
    o_i = nc.dram_tensor("o_i", [DIM, N], F16, kind="ExternalOutput")

    with tile.TileContext(nc) as tc:
        with (
            tc.tile_pool(name="const", bufs=1) as cpool,
            tc.tile_pool(name="work", bufs=2) as pw,
            tc.tile_pool(name="psB", bufs=2, space="PSUM") as psB,
            tc.tile_pool(name="psU", bufs=4, space="PSUM") as psU,
            tc.tile_pool(name="dram", bufs=16, space="DRAM") as pdram,
        ):
            # ---------------- constants ----------------
            hengs = (nc.sync, nc.scalar)
            smask = cpool.tile([128, 1], F32, tag="smask")
            nc.sync.dma_start(smask[:], din["smask"][:, :])
            ident2 = cpool.tile([64, 2, 128], F8, tag="ident2")
            nc.scalar.dma_start(
                ident2[:], bass.AP(din["ident2"], 0,
                                   [[256, 64], [128, 2], [1, 128]]))

            # load order tuned so Q(0)'s inputs land first
            xtt = {}
            qd = 0

            def load_xt(nm):
                nonlocal qd
                t = pw.tile([128, 4, 1024], F16, tag="xt", bufs=2, name=nm)
                hengs[qd % 2].dma_start(
                    t[:], bass.AP(din[nm], 0,
                                  [[N, 128], [128 * N, 4], [1, N]]))
                qd += 1
                xtt[nm] = t

            def xt(nm, kt, nh):
                return xtt[nm][:, kt, nh * 512:(nh + 1) * 512]

            def load_w(nm, tag, bufs):
                # one [128, 4, 512] tile per weight tensor, single DMA
                nonlocal qd
                t = pw.tile([128, 4, 512], F16, tag=tag, bufs=bufs,
                            name=nm)
                hengs[qd % 2].dma_start(
                    t[:], bass.AP(din[nm], 0,
                                  [[512, 128], [128 * 512, 4], [1, 512]]))
                qd += 1
                return [t[:, kt, :] for kt in range(KT)]

            wqa = load_w("wq_a", "wl", 4)
            load_xt("xt_r")
            wqb = load_w("wq_b", "wl", 4)
            load_xt("xt_i")
            rel_r = cpool.tile([128, 2048], F16, tag="rel_r")
            nc.sync.dma_start(rel_r[:], din["rel_r"][:, :])
            wka = load_w("wk_a", "wl", 4)
            wkb = load_w("wk_b", "wl", 4)
            rel_i = cpool.tile([128, 2048], F16, tag="rel_i")
            nc.scalar.dma_start(rel_i[:], din["rel_i"][:, :])
            wva = load_w("wv_a", "wv", 2)
            wvb = load_w("wv_b", "wv", 2)
            wo_re = cpool.tile([128, 4, 512], F16, tag="wo_re")
            wo_im = cpool.tile([128, 4, 512], F16, tag="wo_im")
            nc.sync.dma_start(
                wo_re[:], bass.AP(din["wo_re"], 0,
                                  [[512, 128], [128 * 512, 4], [1, 512]]))
            nc.scalar.dma_start(
                wo_im[:], bass.AP(din["wo_im"], 0,
                                  [[512, 128], [128 * 512, 4], [1, 512]]))

            A = [None] * HPC
            Ai = [None] * HPC
            Knat = [None] * HPC
            Vpp = pw.tile([128, 8, 512], F16, tag="vpp", bufs=1,
                          name="Vpp")

            def emit_proj_unit(kind, h, nh):
                wa, wb = (wqa, wqb) if kind == "q" else (wka, wkb)
                hs = slice(h * 128, (h + 1) * 128)
                ns = slice(nh * 512, (nh + 1) * 512)
                if kind == "q" and A[h] is None:
                    A[h] = pw.tile([128, 1024], F16, tag="stk", bufs=12,
                                   name=f"A{h}")
                    Ai[h] = pw.tile([128, 1024], F16, tag="stk", bufs=12,
                                    name=f"Ai{h}")
                if kind == "k" and Knat[h] is None:
                    Knat[h] = pw.tile([128, 1024], F16, tag="stk",
                                      bufs=12, name=f"Knat{h}")
                ps = psU.tile([128, 512], F32, tag="pu",
                              name=f"ps{kind}_{h}_{nh}")
                for kt in range(KT):
                    nc.tensor.matmul(ps[:], wa[kt][:, hs],
                                     xt("xt_r", kt, nh),
                                     start=(kt == 0), stop=False)
                for kt in range(KT):
                    nc.tensor.matmul(ps[:], wb[kt][:, hs],
                                     xt("xt_i", kt, nh),
                                     start=False, stop=(kt == KT - 1))
                if kind == "q":
                    nc.vector.tensor_scalar_mul(A[h][:, ns], ps[:],
                                                smask[:])
                    # A_i = [qi*s; qr*s] from A = [qr*s; -qi*s]
                    nc.vector.tensor_scalar_mul(Ai[h][0:64, ns],
                                                A[h][64:128, ns], -1.0)
                    nc.vector.tensor_copy(Ai[h][64:128, ns],
                                          A[h][0:64, ns])
                else:
                    nc.scalar.copy(Knat[h][:, ns], ps[:])

            def emit_vproj_unit(J):
                xs = slice((J % 4) * 128, (J % 4) * 128 + 128)
                vps = psU.tile([128, 512], F32, tag="pu", name=f"vps_{J}")
                for kt in range(KT):
                    nc.tensor.matmul(vps[:],
                                     xt("xt_r", kt, J // 4)[:, xs],
                                     wva[kt][:, :],
                                     start=(kt == 0), stop=False)
                for kt in range(KT):
                    nc.tensor.matmul(vps[:],
                                     xt("xt_i", kt, J // 4)[:, xs],
                                     wvb[kt][:, :],
                                     start=False, stop=(kt == KT - 1))
                nc.vector.tensor_copy(Vpp[:, J, :], vps[:])

            # head 0 Q/K up front; the rest feeds the loop's PE slack
            for kind in ("q", "k"):
                for nh in range(2):
                    emit_proj_unit(kind, 0, nh)
            punits = [("q", 1, 0), ("q", 1, 1), ("k", 1, 0), ("k", 1, 1),
                      ("v", 0, None), ("v", 1, None), ("v", 2, None),
                      ("v", 3, None),
                      ("q", 2, 0), ("q", 2, 1), ("k", 2, 0), ("k", 2, 1),
                      ("v", 4, None), ("v", 5, None), ("v", 6, None),
                      ("v", 7, None),
                      ("q", 3, 0), ("q", 3, 1), ("k", 3, 0), ("k", 3, 1)]

            # OT stacks: per-head [avr(64); avi(64)] x n, [128, 1024] fp16
            OT = [pw.tile([128, 1024], F16, tag="otk", bufs=4,
                          name=f"OT{t}") for t in range(4)]

            # ---------------- attention pipeline stages ----------------
            # rel columns e in [512, 1534] vary; e<=511 are all rel_emb[2M]
            # and e>=1535 all rel_emb[0]. Only the varying span + 2 const
            # columns are computed/staged; Pool broadcasts the const
            # regions into qe before the slot write.
            def qwin(I):
                lo = c_lo(I)
                head = max(0, 512 - lo)
                tail = max(0, lo + WIN - 1535)
                return head, WIN - head - tail

            def emit_qrel_part(h, I, part, qe):
                # The const columns (e=511 head / e=1535 tail) adjoin the
                # varying span in e-space, so they ride inside the chunk
                # matmuls; the broadcasts later replicate them in qe.
                isl = slice(I * 128, (I + 1) * 128)
                lo = c_lo(I)
                head, w = qwin(I)
                tail = WIN - head - w
                e0 = lo + head - (1 if head else 0)
                W = w + (1 if head else 0) + (1 if tail else 0)
                q0 = head - (1 if head else 0)
                relt = rel_r if part == 0 else rel_i
                chunks = []
                c = 0
                while c < W:
                    chunks.append((c, min(c + 512, W)))
                    c += 512
                qpss = []
                for ci, (c0, c1) in enumerate(chunks):
                    qps = psU.tile([128, c1 - c0], F32, tag="pu",
                                   name=f"qps{part}_{h}_{I}_{ci}")
                    nc.tensor.matmul(qps[:], A[h][:, isl],
                                     relt[:, e0 + c0:e0 + c1],
                                     start=True, stop=True)
                    qpss.append(qps)
                for ci, (c0, c1) in enumerate(chunks):
                    dst = qe[:, part, q0 + c0:q0 + c1]
                    if ci == 0:
                        nc.vector.tensor_copy(dst, qpss[ci][:])
                    else:
                        nc.scalar.copy(dst, qpss[ci][:])

            def emit_qe_bcast(h, I, qe):
                head, w = qwin(I)
                for part in range(2):
                    if head > 1:
                        nc.gpsimd.tensor_copy(
                            qe[:, part, 0:head - 1],
                            qe[:, part, head - 1:head].broadcast_to(
                                [128, head - 1]))
                    if head + w + 1 < WIN:
                        nc.gpsimd.tensor_copy(
                            qe[:, part, head + w + 1:WIN],
                            qe[:, part, head + w:head + w + 1].broadcast_to(
                                [128, WIN - head - w - 1]))

            def emit_qrel_write(h, I, qe):
                slot = pdram.tile([128, 2 * WIN], F8, tag="qrev",
                                  name=f"qrev_{h}_{I}")
                nc.gpsimd.dma_start(
                    bass.AP(slot.tensor, 0,
                            [[2 * WIN, 128], [WIN, 2], [1, WIN]]),
                    qe[:])
                return slot

            def emit_qrel_read(h, I, slot, mode):
                # part r in DoubleRow layout: (p, ko, j) <- slot row
                # 2p+ko, band col 127-(2p+ko)+j
                if mode == "a":
                    skw = pw.tile([64, 2, 2, 1024], F8, tag="skw", bufs=2,
                                  name=f"skew_{h}_{I}")
                    nc.sync.dma_start(
                        skw[:],
                        bass.AP(slot.tensor, 127,
                                [[2 * (2 * WIN - 1), 64], [2 * WIN - 1, 2],
                                 [WIN, 2], [1, 1024]]))
                    return skw, None
                skwr = pw.tile([64, 2, 1024], F8, tag="skwr", bufs=6,
                               name=f"skewr_{h}_{I}")
                nc.sync.dma_start(
                    skwr[:],
                    bass.AP(slot.tensor, 127,
                            [[2 * (2 * WIN - 1), 64], [2 * WIN - 1, 2],
                             [1, 1024]]))
                # part i natural: (p, j) <- slot row p, col W+127-p+j
                skwi = pw.tile([128, 1024], F8, tag="skwi", bufs=6,
                               name=f"skewi_{h}_{I}")
                nc.sync.dma_start(
                    skwi[:],
                    bass.AP(slot.tensor, WIN + 127,
                            [[2 * WIN - 1, 128], [1, 1024]]))
                return skwr, skwi

            def stage_B(h, I, skws, mode):
                skwr, skwi = skws
                isl = slice(I * 128, (I + 1) * 128)
                # i part first: its SBUF crossing overlaps the r matmuls
                dpsi = psB.tile([128, 1024], F32, tag="pb",
                                name=f"dpsi_{h}_{I}")
                for nh in range(2):
                    ns = slice(nh * 512, (nh + 1) * 512)
                    nc.tensor.matmul(dpsi[:, ns], Ai[h][:, isl],
                                     Knat[h][:, ns], start=True,
                                     stop=(mode == "b"))
                    if mode == "a":
                        nc.tensor.matmul(dpsi[:, ns], ident2[:],
                                         skwr[:, :, 1, ns], start=False,
                                         stop=True, perf_mode=DR)
                ui = pw.tile([128, 1024], F16, tag="ui", bufs=3,
                             name=f"ui_{h}_{I}")
                if mode == "a":
                    nc.scalar.activation(ui[:], dpsi[:], AF.Square)
                else:
                    nc.vector._custom_dve(addsq, out=ui[:],
                                          in0=skwi[:], in1=dpsi[:])
                dpsr = psB.tile([128, 1024], F32, tag="pb",
                                name=f"dpsr_{h}_{I}")
                for nh in range(2):
                    ns = slice(nh * 512, (nh + 1) * 512)
                    nc.tensor.matmul(dpsr[:, ns], A[h][:, isl],
                                     Knat[h][:, ns], start=True, stop=False)
                    sk = skwr[:, :, 0, ns] if mode == "a" else skwr[:, :, ns]
                    nc.tensor.matmul(dpsr[:, ns], ident2[:], sk,
                                     start=False, stop=True, perf_mode=DR)
                m2 = pw.tile([128, 1024], F16, tag="m2", bufs=12,
                             name=f"m2_{h}_{I}")
                nc.vector._custom_dve(sqacc, out=m2[:],
                                      in0=ui[:], in1=dpsr[:])
                return m2

            def emit_sqrt(h, I, m2):
                # in-place: mag overwrites m2
                inst = nc.scalar.activation(m2[:], m2[:], AF.Sqrt)
                return m2, inst

            def emit_exp(h, I, mag, rs8, col):
                attn = pw.tile([128, 1024], F16, tag="attn", bufs=10,
                               name=f"attn_{h}_{I}")
                inst = nc.scalar.activation(attn[:], mag[:], AF.Exp,
                                            accum_out=rs8[:, col:col + 1])
                return attn, inst

            def stage_C1(h, I, attn, rc8, col):
                nc.vector.tensor_scalar_mul(attn[:], attn[:],
                                            rc8[:, col:col + 1])
                return attn

            def stage_C2(h, I, attn, atP):
                half = slice((I % 2) * 128, (I % 2) * 128 + 128)
                nc.sync.dma_start(atP[:, :, half], attn[:], transpose=True)

            def stage_D_pair(h, I0, atP):
                # tiles (h, I0) and (h, I0+1) share one AV matmul pass
                isl = slice(I0 * 128, (I0 + 2) * 128)
                avs = psU.tile([128, 256], F32, tag="pu",
                               name=f"avs_{h}_{I0}")
                vsl = slice(h * 128, (h + 1) * 128)
                for J in range(NT):
                    nc.tensor.matmul(avs[:], Vpp[:, J, vsl],
                                     atP[:, J, :],
                                     start=(J == 0), stop=(J == NT - 1))
                nc.vector.tensor_copy(OT[h][:, isl], avs[:])

            def emit_outproj(nh):
                ns = slice(nh * 512, (nh + 1) * 512)
                for part, wo_s in ((0, wo_re), (1, wo_im)):
                    for dt_ in range(4):
                        ds = slice(dt_ * 128, (dt_ + 1) * 128)
                        ops = psU.tile([128, 512], F32, tag="pu",
                                       name=f"ops_{part}_{dt_}_{nh}")
                        for j in range(4):
                            nc.tensor.matmul(ops[:], wo_s[:, j, ds],
                                             OT[j][:, ns],
                                             start=(j == 0), stop=(j == 3))
                        osb = pw.tile([128, 512], F16, tag="osb", bufs=3,
                                      name=f"osb_{part}_{dt_}_{nh}")
                        nc.scalar.copy(osb[:], ops[:])
                        dst = o_r if part == 0 else o_i
                        nc.sync.dma_start(
                            bass.AP(dst, dt_ * 128 * N + nh * 512,
                                    [[N, 128], [1, 512]]),
                            osb[:])

            flat = [(h, I) for h in range(HPC) for I in range(NT)]
            NF = len(flat)
            (qe_map, qe_done, slotmap, skewmap, m2map, magmap, attnmap,
             atPmap) = ({} for _ in range(8))
            rs8map, rc8map = {}, {}
            for s in range(NF + PD + 1):
                if punits:
                    kind, a1, a2 = punits.pop(0)
                    if kind == "v":
                        emit_vproj_unit(a1)
                    else:
                        emit_proj_unit(kind, a1, a2)
                if s < NF:
                    h, I = flat[s]
                    qe_map[(h, I)] = pw.tile([128, 2, WIN], F8, tag="qe",
                                             bufs=6, name=f"qe_{h}_{I}")
                    emit_qrel_part(h, I, 0, qe_map[(h, I)])
                if PW <= s < NF + PW:
                    h, I = flat[s - PW]
                    slotmap[(h, I)] = emit_qrel_write(h, I,
                                                      qe_done.pop((h, I)))
                if PR <= s < NF + PR:
                    h, I = flat[s - PR]
                    skewmap[(h, I)] = emit_qrel_read(
                        h, I, slotmap.pop((h, I)),
                        "a" if (s - PR) % 8 in (1, 3, 5) else "b")
                # batched Sqrt+Exp (SQG tiles), lagged PL iterations;
                # priority-pushed so later iterations' table-neutral copies
                # interleave instead of stalling behind the burst
                t = s - PB - PL
                if 0 <= t < NF and t % SQG == SQG - 1:
                    g = t // SQG
                    prio0 = tc.cur_priority
                    tc.cur_priority = prio0 + PRIO_BUMP
                    chain = []
                    for tt in range(t - SQG + 1, t + 1):
                        hh, ii = flat[tt]
                        magmap[(hh, ii)], bi = emit_sqrt(
                            hh, ii, m2map.pop((hh, ii)))
                        chain.append(bi)
                    rs8 = pw.tile([128, SQG], F32, tag="sm", bufs=3,
                                  name=f"rs8_{g}")
                    rs8map[g] = rs8
                    for tt in range(t - SQG + 1, t + 1):
                        hh, ii = flat[tt]
                        attnmap[(hh, ii)], bi = emit_exp(
                            hh, ii, magmap.pop((hh, ii)), rs8, tt % SQG)
                        chain.append(bi)
                    # same-engine ordering chain: keeps the batch adjacent
                    # in the ACT queue so only 2 table loads fire per batch
                    import bass_rust as _br
                    for i in range(1, len(chain)):
                        chain[i].ins.add_dependency(
                            chain[i - 1].ins.name,
                            _br.DependencyInfo(sync=False, no_sync=True))
                    rc8 = pw.tile([128, SQG], F32, tag="sm", bufs=3,
                                  name=f"rc8_{g}")
                    nc.vector.reciprocal(rc8[:], rs8[:])
                    rc8map[g] = rc8
                    tc.cur_priority = prio0
                if PC - 1 <= s < NF + PC - 1:
                    h, I = flat[s - PC + 1]
                    t1 = s - PC + 1
                    attnmap[(h, I)] = stage_C1(h, I, attnmap.pop((h, I)),
                                               rc8map[t1 // SQG], t1 % SQG)
                if PC <= s < NF + PC:
                    h, I = flat[s - PC]
                    if I % 2 == 0:
                        atPmap[(h, I // 2)] = pw.tile(
                            [128, 8, 256], F16, tag="att", bufs=3,
                            name=f"atP_{h}_{I // 2}")
                    stage_C2(h, I, attnmap.pop((h, I)), atPmap[(h, I // 2)])
                if PD <= s < NF + PD and (s - PD) % 2 == 1:
                    h, I = flat[s - PD]
                    stage_D_pair(h, I - 1, atPmap.pop((h, I // 2)))
                    if (h, I) == (HPC - 1, 3):
                        emit_outproj(0)
                if PB <= s < NF + PB:
                    h, I = flat[s - PB]
                    m2map[(h, I)] = stage_B(h, I, skewmap.pop((h, I)),
                                            "a" if (s - PB) % 8 in (1, 3, 5)
                                            else "b")
                if s < NF:
                    h, I = flat[s]
                    qe = qe_map.pop((h, I))
                    emit_qrel_part(h, I, 1, qe)
                    emit_qe_bcast(h, I, qe)
                    qe_done[(h, I)] = qe
            emit_outproj(1)

    nc.compile()
    return nc, mag2


def _prep_core_inputs(inputs, core):
    import ml_dtypes
    b, half = core // 2, core % 2
    x = inputs["x"]
    f16 = np.float16
    xt_r = np.ascontiguousarray(x[b, :, :, 0].T).astype(f16)
    xt_i = np.ascontiguousarray(x[b, :, :, 1].T).astype(f16)

    def pack_ab(wr, wi):
        a = np.empty((DIM, 512), f16)
        bb = np.empty((DIM, 512), f16)
        for hl in range(HPC):
            gh = half * HPC + hl
            cs = slice(gh * DH, (gh + 1) * DH)
            a[:, hl * 128:hl * 128 + 64] = wr[:, cs]
            a[:, hl * 128 + 64:hl * 128 + 128] = wi[:, cs]
            bb[:, hl * 128:hl * 128 + 64] = -wi[:, cs]
            bb[:, hl * 128 + 64:hl * 128 + 128] = wr[:, cs]
        return a, bb

    wq_a, wq_b = pack_ab(inputs["wq_r"], inputs["wq_i"])
    wk_a, wk_b = pack_ab(inputs["wkv_r"][:, :512], inputs["wkv_i"][:, :512])
    wv_a, wv_b = pack_ab(inputs["wkv_r"][:, 512:], inputs["wkv_i"][:, 512:])

    # wo rows permuted head-major: per head hl, rows [r(64); i-part(64)]
    rs0 = half * 256
    wo_re = np.empty((DIM, 512), f16)
    wo_im = np.empty((DIM, 512), f16)
    for hl in range(HPC):
        rr = slice(rs0 + hl * 64, rs0 + (hl + 1) * 64)
        dst_r = slice(hl * 128, hl * 128 + 64)
        dst_i = slice(hl * 128 + 64, hl * 128 + 128)
        wo_re[dst_r] = inputs["wo_r"][rr, :]
        wo_re[dst_i] = -inputs["wo_i"][rr, :]
        wo_im[dst_r] = inputs["wo_i"][rr, :]
        wo_im[dst_i] = inputs["wo_r"][rr, :]

    e = np.arange(2047)
    t_ext = inputs["rel_emb"][np.clip(e - 1023, -MAX_POS, MAX_POS) + MAX_POS]
    relrev = t_ext[::-1].astype(np.float32)      # [2047, 64]
    rel_r = np.zeros((128, 2048), f16)
    rel_i = np.zeros((128, 2048), f16)
    rel_r[0:64, 0:2047] = relrev.T.astype(f16)
    rel_i[64:128, 0:2047] = (-relrev.T).astype(f16)

    smask = np.concatenate(
        [np.full(64, SCALE, np.float32),
         np.full(64, -SCALE, np.float32)]).reshape(128, 1)

    ident2 = np.zeros((64, 2, 128), np.float32)
    for p in range(64):
        for k in range(2):
            ident2[p, k, 2 * p + k] = 1.0
    ident2 = ident2.reshape(64, 256).astype(ml_dtypes.float8_e4m3)

    return {
        "xt_r": xt_r, "xt_i": xt_i,
        "wq_a": wq_a, "wq_b": wq_b, "wk_a": wk_a, "wk_b": wk_b,
        "wv_a": wv_a, "wv_b": wv_b, "wo_re": wo_re, "wo_im": wo_im,
        "rel_r": rel_r, "rel_i": rel_i, "smask": smask,
        "ident2": ident2,
    }


_last_results = {}


def kernel(**inputs):
    inputs = {k: np.asarray(v) for k, v in inputs.items()}
    nc, _ = build_module()
    in_maps = [_prep_core_inputs(inputs, c) for c in range(8)]
    res = run_bass_kernel_spmd(nc, in_maps, core_ids=list(range(8)))
    _last_results["res"] = res

    bo_r = inputs["bo_r"].astype(np.float32)
    bo_i = inputs["bo_i"].astype(np.float32)
    out = np.empty((B, N, DIM, 2), np.float32)
    for b in range(B):
        r = (res.results[2 * b]["o_r"].astype(np.float32)
             + res.results[2 * b + 1]["o_r"].astype(np.float32))
        i = (res.results[2 * b]["o_i"].astype(np.float32)
             + res.results[2 * b + 1]["o_i"].astype(np.float32))
        out[b, :, :, 0] = r.T + bo_r[None, :]
        out[b, :, :, 1] = i.T + bo_i[None, :]
    return out
